# revision 3
# baseline (speedup 1.0000x reference)
"""Trainium2 Bass kernel for nn_BallPredictorGNN.

The reference model is a 2-layer GAT over (N=20000, E=640000) followed by an
MLP applied to the LAST node only ("ball") — the output is a single [2] vector.
Only the ball's 2-hop dependency cone matters:

  layer 2 aggregates at the ball node only            (~33 in-edges)
  layer 1 aggregates at the ball's in-neighbours S2   (~33 nodes, ~1100 edges)
  x @ W1 is needed for the sources of those edges S1  (~1100 nodes)

Host side (pure data routing): extract the cone, sort edges by destination,
build padded index tables.  Device side (all FLOPs): dense projections via
TensorE, per-edge gathers via indirect DMA, segment-softmax-aggregation via
one-hot matmuls with fused numerator/denominator accumulation in PSUM
(alpha = exp(e)/sum exp(e) folded as  out = (sum exp(e)*h_src) / sum exp(e)).

The same program is replicated SPMD on all 8 NeuronCores (the cone is tiny, so
replication beats sharding + collectives); core 0's output is returned.
"""

import numpy as np

P = 128
_CACHE = {}


def _ceil(a, b):
    return -(-a // b)


def _pad_rows(a, n, fill=0):
    out = np.full((n,) + a.shape[1:], fill, a.dtype)
    out[: len(a)] = a
    return out


def _host_preprocess(inputs):
    x = np.ascontiguousarray(np.asarray(inputs["x"], dtype=np.float32))
    ei = np.asarray(inputs["edge_index"]).astype(np.int64)
    N = x.shape[0]
    F = x.shape[1]
    ball = N - 1
    src, dst = ei[0], ei[1]

    # ---- layer-2 cone: edges into the ball (+ self loop) --------------------
    sel2 = dst == ball
    e2s = np.concatenate([src[sel2], [ball]])
    uniq = np.unique(e2s)
    S2 = np.concatenate([[ball], uniq[uniq != ball]]).astype(np.int64)
    m2 = len(S2)
    assert m2 <= 127, f"ball in-neighbourhood too large for one dst block: {m2}"

    # ---- layer-1 cone: edges into S2 (+ self loops for S2) ------------------
    in_S2 = np.zeros(N, dtype=bool)
    in_S2[S2] = True
    sel1 = in_S2[dst]
    e1s = np.concatenate([src[sel1], S2])
    e1d = np.concatenate([dst[sel1], S2])
    uniq1 = np.unique(e1s)
    rest = uniq1[~in_S2[uniq1]]
    S1 = np.concatenate([S2, rest])  # S2 is a prefix of S1
    m1 = len(S1)
    m1p = _ceil(m1, P) * P

    loc1 = np.full(N, -1, dtype=np.int64)
    loc1[S1] = np.arange(m1)
    s_loc = loc1[e1s]
    d_loc = loc1[e1d]  # in [0, m2)
    order = np.argsort(d_loc, kind="stable")
    s_loc, d_loc = s_loc[order], d_loc[order]
    n1 = len(s_loc)
    T1 = _ceil(n1, P)
    n1p = T1 * P

    # per-tile column layout [P, T]: element (p, t) = edge t*P + p
    def cols(a, n_pad, fill, dt):
        return np.ascontiguousarray(
            _pad_rows(a.astype(dt), n_pad, fill).reshape(-1, P).T
        )

    src1 = cols(s_loc, n1p, 0, np.int32)
    dst1 = cols(d_loc, n1p, 0, np.int32)
    dstrel1 = cols(d_loc, n1p, m2, np.float32)  # pad edges -> garbage slot m2

    s2_loc = loc1[e2s]  # all < m2
    n2 = len(s2_loc)
    T2 = _ceil(n2, P)
    n2p = T2 * P
    src2 = cols(s2_loc, n2p, 0, np.int32)
    dst2 = cols(np.zeros(n2, np.int64), n2p, 0, np.int32)
    dstrel2 = cols(np.zeros(n2, np.int64), n2p, 1, np.float32)

    # ---- dense operands -----------------------------------------------------
    xT = np.ascontiguousarray(_pad_rows(x[S1], m1p).T)  # [F, m1p]

    W1 = np.asarray(inputs["W1"], np.float32)  # [F, 4*64]
    a_src1 = np.asarray(inputs["a_src1"], np.float32)  # [4, 64]
    a_dst1 = np.asarray(inputs["a_dst1"], np.float32)
    H1, C = a_src1.shape
    ablk = np.zeros((H1 * C, 2 * H1), np.float32)  # [256, 8] = [Ad | As]
    for h in range(H1):
        ablk[h * C : (h + 1) * C, h] = a_dst1[h]
        ablk[h * C : (h + 1) * C, H1 + h] = a_src1[h]

    W2 = np.asarray(inputs["W2"], np.float32)  # [256, 64]
    a2 = np.stack(
        [np.asarray(inputs["a_dst2"], np.float32)[0],
         np.asarray(inputs["a_src2"], np.float32)[0]],
        axis=1,
    )  # [64, 2] = [a_dst | a_src]

    feed = {
        "xT": xT,
        "w1": np.ascontiguousarray(W1),
        "w1T": np.ascontiguousarray(W1.T),
        "ablk": ablk,
        "b1bc": np.ascontiguousarray(
            np.broadcast_to(np.asarray(inputs["b1"], np.float32), (P, H1 * C))
        ),
        "w2": np.ascontiguousarray(W2),
        "w2T": np.ascontiguousarray(W2.T),
        "a2": a2,
        "b2row": np.asarray(inputs["b2"], np.float32)[None, :].copy(),
        "fc1w": np.ascontiguousarray(np.asarray(inputs["fc1_w"], np.float32)),
        "fc1b": np.asarray(inputs["fc1_b"], np.float32)[:, None].copy(),
        "fc2w": np.ascontiguousarray(np.asarray(inputs["fc2_w"], np.float32)),
        "fc2b": np.asarray(inputs["fc2_b"], np.float32)[:, None].copy(),
        "src1": src1,
        "dst1": dst1,
        "dstrel1": dstrel1,
        "src2": src2,
        "dst2": dst2,
        "dstrel2": dstrel2,
    }
    dims = dict(F=F, H1=H1, C=C, m1p=m1p, m2=m2, T1=T1, T2=T2)
    return feed, dims


def _build(dims):
    from concourse import bacc, bass, mybir, tile
    from concourse.masks import make_identity

    F = dims["F"]          # 128 input features
    H1 = dims["H1"]        # 4 heads, layer 1
    C = dims["C"]          # 64 channels per head
    D1 = H1 * C            # 256
    G1W = 2 * H1 + D1      # 264 = [ad(4) | as(4) | h(256)]
    G2W = 2 + C            # 66  = [ad2 | as2 | h2p]
    m1p, m2, T1, T2 = dims["m1p"], dims["m2"], dims["T1"], dims["T2"]
    NCH1 = m1p // P
    f32 = mybir.dt.float32
    i32 = mybir.dt.int32
    AX0 = lambda ap: bass.IndirectOffsetOnAxis(ap=ap, axis=0)

    nc = bacc.Bacc("TRN2", target_bir_lowering=False, debug=False)

    xT_d = nc.declare_dram_parameter("xT", [F, m1p], f32, isOutput=False)
    w1_d = nc.declare_dram_parameter("w1", [F, D1], f32, isOutput=False)
    w1T_d = nc.declare_dram_parameter("w1T", [D1, F], f32, isOutput=False)
    ablk_d = nc.declare_dram_parameter("ablk", [D1, 2 * H1], f32, isOutput=False)
    b1bc_d = nc.declare_dram_parameter("b1bc", [P, D1], f32, isOutput=False)
    w2_d = nc.declare_dram_parameter("w2", [D1, C], f32, isOutput=False)
    w2T_d = nc.declare_dram_parameter("w2T", [C, D1], f32, isOutput=False)
    a2_d = nc.declare_dram_parameter("a2", [C, 2], f32, isOutput=False)
    b2_d = nc.declare_dram_parameter("b2row", [1, C], f32, isOutput=False)
    fc1w_d = nc.declare_dram_parameter("fc1w", [C, C // 2], f32, isOutput=False)
    fc1b_d = nc.declare_dram_parameter("fc1b", [C // 2, 1], f32, isOutput=False)
    fc2w_d = nc.declare_dram_parameter("fc2w", [C // 2, 2], f32, isOutput=False)
    fc2b_d = nc.declare_dram_parameter("fc2b", [2, 1], f32, isOutput=False)
    src1_d = nc.declare_dram_parameter("src1", [P, T1], i32, isOutput=False)
    dst1_d = nc.declare_dram_parameter("dst1", [P, T1], i32, isOutput=False)
    drel1_d = nc.declare_dram_parameter("dstrel1", [P, T1], f32, isOutput=False)
    src2_d = nc.declare_dram_parameter("src2", [P, T2], i32, isOutput=False)
    dst2_d = nc.declare_dram_parameter("dst2", [P, T2], i32, isOutput=False)
    drel2_d = nc.declare_dram_parameter("dstrel2", [P, T2], f32, isOutput=False)
    out_d = nc.declare_dram_parameter("out", [2, 1], f32, isOutput=True)

    g1_d = nc.dram_tensor("g1_tab", [m1p, G1W], f32)
    ad1_d = nc.dram_tensor("ad1_tab", [m1p, 2 * H1], f32)
    g2_d = nc.dram_tensor("g2_tab", [P, G2W], f32)
    ad2_d = nc.dram_tensor("ad2_tab", [P, 2], f32)

    EQ = mybir.AluOpType.is_equal
    MAX = mybir.AluOpType.max
    Copy = mybir.ActivationFunctionType.Copy
    Exp = mybir.ActivationFunctionType.Exp
    Relu = mybir.ActivationFunctionType.Relu

    with tile.TileContext(nc) as tc:
        with (
            tc.tile_pool(name="const", bufs=1) as cp,
            tc.tile_pool(name="work", bufs=3) as wp,
            tc.tile_pool(name="fin", bufs=1) as fp,
            tc.tile_pool(name="psum", bufs=2, space="PSUM") as pp,
            tc.tile_pool(name="acc", bufs=1, space="PSUM") as ap_,
        ):
            # ---------------- constants / inputs into SBUF ----------------
            ident = cp.tile([P, P], f32)
            make_identity(nc, ident[:])
            iota_i = cp.tile([P, P], i32)
            nc.gpsimd.iota(iota_i[:], pattern=[[1, P]], base=0, channel_multiplier=0)
            iota_f = cp.tile([P, P], f32)
            nc.vector.tensor_copy(iota_f[:], iota_i[:])

            xT_s = cp.tile([F, m1p], f32)
            nc.sync.dma_start(xT_s[:], xT_d[:])
            rhs1 = cp.tile([F, G1W], f32)
            nc.sync.dma_start(rhs1[:, 2 * H1 :], w1_d[:])
            b1bc = cp.tile([P, D1], f32)
            nc.sync.dma_start(b1bc[:], b1bc_d[:])
            sr1 = cp.tile([P, T1], i32)
            nc.sync.dma_start(sr1[:], src1_d[:])
            ds1 = cp.tile([P, T1], i32)
            nc.sync.dma_start(ds1[:], dst1_d[:])
            dr1 = cp.tile([P, T1], f32)
            nc.sync.dma_start(dr1[:], drel1_d[:])
            sr2 = cp.tile([P, T2], i32)
            nc.sync.dma_start(sr2[:], src2_d[:])
            ds2 = cp.tile([P, T2], i32)
            nc.sync.dma_start(ds2[:], dst2_d[:])
            dr2 = cp.tile([P, T2], f32)
            nc.sync.dma_start(dr2[:], drel2_d[:])
            w2T_s = cp.tile([C, D1], f32)
            nc.sync.dma_start(w2T_s[:], w2T_d[:])
            a2_s = cp.tile([C, 2], f32)
            nc.sync.dma_start(a2_s[:], a2_d[:])
            b2_s = cp.tile([1, C], f32)
            nc.sync.dma_start(b2_s[:], b2_d[:])
            fc1w_s = cp.tile([C, C // 2], f32)
            nc.sync.dma_start(fc1w_s[:], fc1w_d[:])
            fc1b_s = cp.tile([C // 2, 1], f32)
            nc.sync.dma_start(fc1b_s[:], fc1b_d[:])
            fc2w_s = cp.tile([C // 2, 2], f32)
            nc.sync.dma_start(fc2w_s[:], fc2w_d[:])
            fc2b_s = cp.tile([2, 1], f32)
            nc.sync.dma_start(fc2b_s[:], fc2b_d[:])

            # ---------------- W1 @ [Ad | As]  (K = 256, 2 chunks) ----------
            pwa = pp.tile([F, 2 * H1], f32, tag="mm")
            for k in range(D1 // P):
                w1T_k = wp.tile([P, F], f32, tag="w1Tk")
                nc.sync.dma_start(w1T_k[:], w1T_d[k * P : (k + 1) * P, :])
                ablk_k = wp.tile([P, 2 * H1], f32, tag="ablkk")
                nc.sync.dma_start(ablk_k[:], ablk_d[k * P : (k + 1) * P, :])
                nc.tensor.matmul(
                    out=pwa[:], lhsT=w1T_k[:], rhs=ablk_k[:],
                    start=(k == 0), stop=(k == D1 // P - 1),
                )
            nc.vector.tensor_copy(rhs1[:, : 2 * H1], pwa[:])

            # ---------------- G1 rows: [ad | as | h] = x @ [W1Ad|W1As|W1] --
            for ci in range(NCH1):
                pg = pp.tile([P, G1W], f32, tag="pg")
                nc.tensor.matmul(
                    out=pg[:], lhsT=xT_s[:, ci * P : (ci + 1) * P], rhs=rhs1[:],
                    start=True, stop=True,
                )
                gsb = wp.tile([P, G1W], f32, tag="gsb")
                nc.scalar.copy(gsb[:], pg[:])
                nc.sync.dma_start(g1_d[ci * P : (ci + 1) * P, :], gsb[:])
                nc.sync.dma_start(ad1_d[ci * P : (ci + 1) * P, :], gsb[:, : 2 * H1])

            # ---------------- layer-1 edge aggregation ---------------------
            agg1 = ap_.tile([P, D1 + H1], f32, tag="agg1")
            for t in range(T1):
                gs = wp.tile([P, G1W], f32, tag="gs")
                nc.gpsimd.indirect_dma_start(
                    out=gs[:], out_offset=None, in_=g1_d[:, :],
                    in_offset=AX0(sr1[:, t : t + 1]),
                )
                adt = wp.tile([P, 2 * H1], f32, tag="adt")
                nc.gpsimd.indirect_dma_start(
                    out=adt[:], out_offset=None, in_=ad1_d[:, :],
                    in_offset=AX0(ds1[:, t : t + 1]),
                )
                e = wp.tile([P, H1], f32, tag="e")
                nc.vector.tensor_add(e[:], gs[:, H1 : 2 * H1], adt[:, :H1])
                es = wp.tile([P, H1], f32, tag="es")
                nc.vector.tensor_scalar_mul(es[:], e[:], 0.2)
                el = wp.tile([P, H1], f32, tag="el")
                nc.vector.tensor_tensor(out=el[:], in0=e[:], in1=es[:], op=MAX)
                pe = wp.tile([P, H1], f32, tag="pe")
                nc.scalar.activation(pe[:], el[:], Exp)
                rhs_t = wp.tile([P, D1 + H1], f32, tag="rhs_t")
                for h in range(H1):
                    sl = slice(h * C, (h + 1) * C)
                    gsl = slice(2 * H1 + h * C, 2 * H1 + (h + 1) * C)
                    if h % 2 == 0:
                        nc.scalar.activation(
                            rhs_t[:, sl], gs[:, gsl], Copy, scale=pe[:, h : h + 1]
                        )
                    else:
                        nc.vector.tensor_scalar_mul(
                            rhs_t[:, sl], gs[:, gsl], pe[:, h : h + 1]
                        )
                nc.vector.tensor_copy(rhs_t[:, D1:], pe[:])
                em = wp.tile([P, P], f32, tag="em")
                nc.vector.tensor_scalar(
                    em[:], iota_f[:], dr1[:, t : t + 1], None, EQ
                )
                nc.tensor.matmul(
                    out=agg1[:], lhsT=em[:], rhs=rhs_t[:],
                    start=(t == 0), stop=(t == T1 - 1),
                )

            # ---------------- layer-1 finalize: h1r = relu(num/den + b1) ---
            den1 = fp.tile([P, H1], f32)
            nc.vector.tensor_scalar_add(den1[:], agg1[:, D1:], 1e-16)
            rec1 = fp.tile([P, H1], f32)
            nc.vector.reciprocal(rec1[:], den1[:])
            h1t = fp.tile([P, D1], f32)
            for h in range(H1):
                sl = slice(h * C, (h + 1) * C)
                nc.scalar.activation(
                    h1t[:, sl], agg1[:, sl], Copy, scale=rec1[:, h : h + 1]
                )
            h1b = fp.tile([P, D1], f32)
            nc.vector.tensor_add(h1b[:], h1t[:], b1bc[:])
            h1r = fp.tile([P, D1], f32)
            nc.scalar.activation(h1r[:], h1b[:], Relu)

            # ---------------- layer-2 projection: G2 = [ad2|as2|h2p] -------
            pg2 = ap_.tile([P, G2W], f32, tag="pg2")
            for k in range(D1 // P):
                ptr = pp.tile([P, P], f32, tag="mm")
                nc.tensor.transpose(
                    out=ptr[:], in_=h1r[:, k * P : (k + 1) * P], identity=ident[:]
                )
                h1rT_k = wp.tile([P, P], f32, tag="h1rTk")
                nc.vector.tensor_copy(h1rT_k[:], ptr[:])
                pwa2 = pp.tile([P, 2], f32, tag="mm")
                nc.tensor.matmul(
                    out=pwa2[:], lhsT=w2T_s[:, k * P : (k + 1) * P], rhs=a2_s[:],
                    start=True, stop=True,
                )
                rhs2_k = wp.tile([P, G2W], f32, tag="rhs2k")
                nc.vector.tensor_copy(rhs2_k[:, :2], pwa2[:])
                nc.sync.dma_start(rhs2_k[:, 2:], w2_d[k * P : (k + 1) * P, :])
                nc.tensor.matmul(
                    out=pg2[:], lhsT=h1rT_k[:], rhs=rhs2_k[:],
                    start=(k == 0), stop=(k == D1 // P - 1),
                )
            g2sb = fp.tile([P, G2W], f32)
            nc.scalar.copy(g2sb[:], pg2[:])
            nc.sync.dma_start(g2_d[:], g2sb[:])
            nc.sync.dma_start(ad2_d[:], g2sb[:, :2])

            # ---------------- layer-2 edge aggregation (ball only) ---------
            agg2 = ap_.tile([P, C + 1], f32, tag="agg2")
            for t in range(T2):
                gs2 = wp.tile([P, G2W], f32, tag="gs2")
                nc.gpsimd.indirect_dma_start(
                    out=gs2[:], out_offset=None, in_=g2_d[:, :],
                    in_offset=AX0(sr2[:, t : t + 1]),
                )
                adt2 = wp.tile([P, 2], f32, tag="adt2")
                nc.gpsimd.indirect_dma_start(
                    out=adt2[:], out_offset=None, in_=ad2_d[:, :],
                    in_offset=AX0(ds2[:, t : t + 1]),
                )
                e2 = wp.tile([P, 1], f32, tag="e2")
                nc.vector.tensor_add(e2[:], gs2[:, 1:2], adt2[:, :1])
                es2 = wp.tile([P, 1], f32, tag="es2")
                nc.vector.tensor_scalar_mul(es2[:], e2[:], 0.2)
                el2 = wp.tile([P, 1], f32, tag="el2")
                nc.vector.tensor_tensor(out=el2[:], in0=e2[:], in1=es2[:], op=MAX)
                pe2 = wp.tile([P, 1], f32, tag="pe2")
                nc.scalar.activation(pe2[:], el2[:], Exp)
                rhs2t = wp.tile([P, C + 1], f32, tag="rhs2t")
                nc.scalar.activation(
                    rhs2t[:, :C], gs2[:, 2:], Copy, scale=pe2[:, 0:1]
                )
                nc.vector.tensor_copy(rhs2t[:, C:], pe2[:])
                em2 = wp.tile([P, P], f32, tag="em2")
                nc.vector.tensor_scalar(
                    em2[:], iota_f[:], dr2[:, t : t + 1], None, EQ
                )
                nc.tensor.matmul(
                    out=agg2[:], lhsT=em2[:], rhs=rhs2t[:],
                    start=(t == 0), stop=(t == T2 - 1),
                )

            # ---------------- ball finalize + MLP --------------------------
            den2 = fp.tile([1, 1], f32)
            nc.vector.tensor_scalar_add(den2[:], agg2[0:1, C : C + 1], 1e-16)
            rec2 = fp.tile([1, 1], f32)
            nc.vector.reciprocal(rec2[:], den2[:])
            bf = fp.tile([1, C], f32)
            nc.scalar.activation(bf[:], agg2[0:1, :C], Copy, scale=rec2[:, 0:1])
            bfb = fp.tile([1, C], f32)
            nc.vector.tensor_add(bfb[:], bf[:], b2_s[:])
            bfr = fp.tile([1, C], f32)
            nc.scalar.activation(bfr[:], bfb[:], Relu)

            ptb = pp.tile([C, 1], f32, tag="mm")
            nc.tensor.transpose(out=ptb[:], in_=bfr[:], identity=ident[0:1, 0:1])
            bcol = fp.tile([C, 1], f32)
            nc.vector.tensor_copy(bcol[:], ptb[:])

            pz = pp.tile([C // 2, 1], f32, tag="mm")
            nc.tensor.matmul(out=pz[:], lhsT=fc1w_s[:], rhs=bcol[:],
                             start=True, stop=True)
            zb = fp.tile([C // 2, 1], f32)
            nc.vector.tensor_add(zb[:], pz[:], fc1b_s[:])
            zr = fp.tile([C // 2, 1], f32)
            nc.scalar.activation(zr[:], zb[:], Relu)

            po = pp.tile([2, 1], f32, tag="mm")
            nc.tensor.matmul(out=po[:], lhsT=fc2w_s[:], rhs=zr[:],
                             start=True, stop=True)
            osb = fp.tile([2, 1], f32)
            nc.vector.tensor_add(osb[:], po[:], fc2b_s[:])
            nc.sync.dma_start(out_d[:], osb[:])

    nc.compile()
    return nc


def kernel(**inputs):
    from concourse.bass_utils import run_bass_kernel_spmd

    feed, dims = _host_preprocess(inputs)
    key = tuple(sorted(dims.items()))
    if key not in _CACHE:
        _CACHE[key] = _build(dims)
    nc = _CACHE[key]

    n_cores = 8
    in_maps = [dict(feed) for _ in range(n_cores)]
    res = run_bass_kernel_spmd(nc, in_maps, core_ids=list(range(n_cores)))
    out = np.asarray(res.results[0]["out"], dtype=np.float32).reshape(2)
    return out


# revision 6
# speedup vs baseline: 1.2633x; 1.2633x over previous
"""Trainium2 Bass kernel for nn_BallPredictorGNN.

The reference model is a 2-layer GAT over (N=20000, E=640000) followed by an
MLP applied to the LAST node only ("ball") — the output is a single [2] vector.
Only the ball's 2-hop dependency cone matters:

  layer 2 aggregates at the ball node only            (~25 in-edges)
  layer 1 aggregates at the ball's in-neighbours S2   (~25 nodes, ~800 edges)
  x @ W1 is needed for the sources of those edges S1  (~800 nodes)

Host side (pure data routing): extract the cone, sort edges by destination,
build padded index tables, pack small operands.  Device side (all FLOPs):
dense projections via TensorE, per-edge-tile row gathers via indirect DMA,
segment-softmax-aggregation via one-hot matmuls with fused numerator /
denominator accumulation in PSUM
(alpha = exp(e)/sum exp(e) folded as  out = (sum exp(e)*h_src) / sum exp(e)).
The per-edge a_dst[dst] term is expanded on-chip as Ematᵀ.T @ ad_slots
(one PE transpose + one small matmul) instead of a second gather, keeping the
serialized GpSimd indirect-DMA queue to one gather per 128 edges.

The same program is replicated SPMD on all 8 NeuronCores (the cone is tiny, so
replication beats sharding + collectives); core 0's output is returned.
"""

import numpy as np

P = 128
_CACHE = {}


def _ceil(a, b):
    return -(-a // b)


def _pad_rows(a, n, fill=0):
    out = np.full((n,) + a.shape[1:], fill, a.dtype)
    out[: len(a)] = a
    return out


class _Packer:
    """Pack many small [p, w] operands into one [128, W] array, column-wise."""

    def __init__(self):
        self.cols = []
        self.pos = 0
        self.slots = {}

    def add(self, name, arr):
        p, w = arr.shape
        full = np.zeros((P, w), arr.dtype)
        full[:p] = arr
        self.cols.append(full)
        self.slots[name] = (self.pos, self.pos + w)
        self.pos += w

    def finish(self):
        return np.ascontiguousarray(np.concatenate(self.cols, axis=1))


def _host_preprocess(inputs):
    x = np.ascontiguousarray(np.asarray(inputs["x"], dtype=np.float32))
    ei = np.asarray(inputs["edge_index"]).astype(np.int64)
    N = x.shape[0]
    F = x.shape[1]
    ball = N - 1
    src, dst = ei[0], ei[1]

    # ---- layer-2 cone: edges into the ball (+ self loop) --------------------
    sel2 = dst == ball
    e2s = np.concatenate([src[sel2], [ball]])
    uniq = np.unique(e2s)
    S2 = np.concatenate([[ball], uniq[uniq != ball]]).astype(np.int64)
    m2 = len(S2)
    assert m2 <= 127, f"ball in-neighbourhood too large for one dst block: {m2}"

    # ---- layer-1 cone: edges into S2 (+ self loops for S2) ------------------
    in_S2 = np.zeros(N, dtype=bool)
    in_S2[S2] = True
    sel1 = in_S2[dst]
    e1s = np.concatenate([src[sel1], S2])
    e1d = np.concatenate([dst[sel1], S2])
    uniq1 = np.unique(e1s)
    rest = uniq1[~in_S2[uniq1]]
    S1 = np.concatenate([S2, rest])  # S2 is a prefix of S1
    m1 = len(S1)
    m1p = _ceil(m1, P) * P

    loc1 = np.full(N, -1, dtype=np.int64)
    loc1[S1] = np.arange(m1)
    s_loc = loc1[e1s]
    d_loc = loc1[e1d]  # in [0, m2)
    order = np.argsort(d_loc, kind="stable")
    s_loc, d_loc = s_loc[order], d_loc[order]
    n1 = len(s_loc)
    T1 = _ceil(n1, P)
    n1p = T1 * P

    # per-tile column layout [P, T]: element (p, t) = edge t*P + p
    def cols(a, n_pad, fill, dt):
        return np.ascontiguousarray(
            _pad_rows(a.astype(dt), n_pad, fill).reshape(-1, P).T
        )

    s2_loc = loc1[e2s]  # all < m2
    n2 = len(s2_loc)
    T2 = _ceil(n2, P)
    n2p = T2 * P

    pki = _Packer()
    pki.add("src1", cols(s_loc, n1p, 0, np.int32))
    pki.add("src2", cols(s2_loc, n2p, 0, np.int32))

    # ---- dense operands -----------------------------------------------------
    xT = np.ascontiguousarray(_pad_rows(x[S1], m1p).T)  # [F, m1p]

    W1 = np.asarray(inputs["W1"], np.float32)  # [F, 4*64]
    a_src1 = np.asarray(inputs["a_src1"], np.float32)  # [4, 64]
    a_dst1 = np.asarray(inputs["a_dst1"], np.float32)
    H1, C = a_src1.shape
    D1 = H1 * C
    ablk = np.zeros((D1, 2 * H1), np.float32)  # [256, 8] = [Ad | As]
    for h in range(H1):
        ablk[h * C : (h + 1) * C, h] = a_dst1[h]
        ablk[h * C : (h + 1) * C, H1 + h] = a_src1[h]

    W2 = np.asarray(inputs["W2"], np.float32)  # [256, 64]
    a2 = np.stack(
        [np.asarray(inputs["a_dst2"], np.float32)[0],
         np.asarray(inputs["a_src2"], np.float32)[0]],
        axis=1,
    )  # [64, 2] = [a_dst | a_src]

    pkf = _Packer()
    pkf.add("w1", W1)
    W1T = np.ascontiguousarray(W1.T)
    for k in range(D1 // P):
        pkf.add(f"w1T{k}", W1T[k * P : (k + 1) * P])
        pkf.add(f"ablk{k}", ablk[k * P : (k + 1) * P])
        pkf.add(f"w2_{k}", W2[k * P : (k + 1) * P])
    pkf.add("b1bc", np.broadcast_to(np.asarray(inputs["b1"], np.float32), (P, D1)))
    pkf.add("w2T", np.ascontiguousarray(W2.T))
    pkf.add("a2", a2)
    pkf.add("b2row", np.asarray(inputs["b2"], np.float32)[None, :])
    pkf.add("fc1w", np.ascontiguousarray(np.asarray(inputs["fc1_w"], np.float32)))
    pkf.add("fc1b", np.asarray(inputs["fc1_b"], np.float32)[:, None])
    pkf.add("fc2w", np.ascontiguousarray(np.asarray(inputs["fc2_w"], np.float32)))
    pkf.add("fc2b", np.asarray(inputs["fc2_b"], np.float32)[:, None])
    pkf.add("dstrel1", cols(d_loc, n1p, m2, np.float32))  # pad -> garbage slot m2
    pkf.add("dstrel2", cols(np.zeros(n2, np.int64), n2p, 1, np.float32))

    feed = {"xT": xT, "packf": pkf.finish(), "packi": pki.finish()}
    dims = dict(
        F=F, H1=H1, C=C, m1p=m1p, m2=m2, T1=T1, T2=T2,
        slots_f=tuple(sorted(pkf.slots.items())),
        slots_i=tuple(sorted(pki.slots.items())),
    )
    return feed, dims


def _build(dims):
    from concourse import bacc, bass, mybir, tile
    from concourse.masks import make_identity

    F = dims["F"]          # 128 input features
    H1 = dims["H1"]        # 4 heads, layer 1
    C = dims["C"]          # 64 channels per head
    D1 = H1 * C            # 256
    G1W = 2 * H1 + D1      # 264 = [ad(4) | as(4) | h(256)]
    G2W = 2 + C            # 66  = [ad2 | as2 | h2p]
    m1p, m2, T1, T2 = dims["m1p"], dims["m2"], dims["T1"], dims["T2"]
    NCH1 = m1p // P
    KCH = D1 // P          # 2 contraction chunks over 256
    slots_f = dict(dims["slots_f"])
    slots_i = dict(dims["slots_i"])
    WF = max(b for _, b in slots_f.values())
    WI = max(b for _, b in slots_i.values())
    f32 = mybir.dt.float32
    i32 = mybir.dt.int32
    AX0 = lambda ap: bass.IndirectOffsetOnAxis(ap=ap, axis=0)

    nc = bacc.Bacc("TRN2", target_bir_lowering=False, debug=False)

    xT_d = nc.declare_dram_parameter("xT", [F, m1p], f32, isOutput=False)
    pf_d = nc.declare_dram_parameter("packf", [P, WF], f32, isOutput=False)
    pi_d = nc.declare_dram_parameter("packi", [P, WI], i32, isOutput=False)
    out_d = nc.declare_dram_parameter("out", [2, 1], f32, isOutput=True)

    g1_d = nc.dram_tensor("g1_tab", [m1p, G1W], f32)
    g2_d = nc.dram_tensor("g2_tab", [P, G2W], f32)

    EQ = mybir.AluOpType.is_equal
    MAX = mybir.AluOpType.max
    Copy = mybir.ActivationFunctionType.Copy
    Exp = mybir.ActivationFunctionType.Exp
    Relu = mybir.ActivationFunctionType.Relu

    with tile.TileContext(nc) as tc:
        with (
            tc.tile_pool(name="const", bufs=1) as cp,
            tc.tile_pool(name="work", bufs=3) as wp,
            tc.tile_pool(name="fin", bufs=1) as fp,
            tc.tile_pool(name="psum", bufs=2, space="PSUM") as pp,
            tc.tile_pool(name="acc", bufs=1, space="PSUM") as ap_,
        ):
            # ---------------- constants / inputs into SBUF ----------------
            pk = cp.tile([P, WF], f32)
            nc.sync.dma_start(pk[:], pf_d[:])
            pki_s = cp.tile([P, WI], i32)
            nc.sync.dma_start(pki_s[:], pi_d[:])
            xT_s = cp.tile([F, m1p], f32)
            nc.sync.dma_start(xT_s[:], xT_d[:])

            def fsl(name, rows=P):
                a, b = slots_f[name]
                return pk[:rows, a:b]

            def isl(name):
                a, b = slots_i[name]
                return pki_s[:, a:b]

            ident = cp.tile([P, P], f32)
            make_identity(nc, ident[:])
            iota_f = cp.tile([P, P], f32)
            nc.gpsimd.iota(
                iota_f[:], pattern=[[1, P]], base=0, channel_multiplier=0,
                allow_small_or_imprecise_dtypes=True,
            )

            # ---------------- W1 @ [Ad | As]  (K = 256, 2 chunks) ----------
            pwa = pp.tile([F, 2 * H1], f32, tag="mm")
            for k in range(KCH):
                nc.tensor.matmul(
                    out=pwa[:], lhsT=fsl(f"w1T{k}"), rhs=fsl(f"ablk{k}"),
                    start=(k == 0), stop=(k == KCH - 1),
                )
            rhs1 = cp.tile([F, G1W], f32)
            nc.vector.tensor_copy(rhs1[:, : 2 * H1], pwa[:])
            nc.scalar.copy(rhs1[:, 2 * H1 :], fsl("w1"))

            # -------- G1 rows: [ad | as | h] = x @ [W1Ad | W1As | W1] ------
            gall = fp.tile([P, NCH1 * G1W], f32)
            for ci in range(NCH1):
                pg = pp.tile([P, G1W], f32, tag="pg")
                nc.tensor.matmul(
                    out=pg[:], lhsT=xT_s[:, ci * P : (ci + 1) * P], rhs=rhs1[:],
                    start=True, stop=True,
                )
                if ci % 2:
                    nc.scalar.copy(gall[:, ci * G1W : (ci + 1) * G1W], pg[:])
                else:
                    nc.vector.tensor_copy(gall[:, ci * G1W : (ci + 1) * G1W], pg[:])
            gall3 = gall[:].rearrange("p (c w) -> p c w", w=G1W)
            nc.sync.dma_start(
                g1_d[:].rearrange("(c p) w -> p c w", p=P), gall3
            )
            # a_dst values for the destination slots: S2 is a prefix of S1,
            # so slot s lives in pre-phase chunk 0, row s, cols 0:H1.
            ad_slots = gall[:, 0:H1]

            # ---------------- layer-1 edge aggregation ---------------------
            agg1 = ap_.tile([P, D1 + H1], f32, tag="agg1")
            drel1 = fsl("dstrel1")
            src1 = isl("src1")
            for t in range(T1):
                gs = wp.tile([P, G1W], f32, tag="gs")
                nc.gpsimd.indirect_dma_start(
                    out=gs[:], out_offset=None, in_=g1_d[:, :],
                    in_offset=AX0(src1[:, t : t + 1]),
                )
                em = wp.tile([P, P], f32, tag="em")
                nc.vector.tensor_scalar(
                    em[:], iota_f[:], drel1[:, t : t + 1], None, EQ
                )
                emt_p = pp.tile([P, P], f32, tag="mm")
                nc.tensor.transpose(out=emt_p[:], in_=em[:], identity=ident[:])
                emt = wp.tile([P, P], f32, tag="emt")
                nc.scalar.copy(emt[:], emt_p[:])
                adx_p = pp.tile([P, H1], f32, tag="mm")
                nc.tensor.matmul(
                    out=adx_p[:], lhsT=emt[:], rhs=ad_slots,
                    start=True, stop=True,
                )
                e = wp.tile([P, H1], f32, tag="e")
                nc.vector.tensor_add(e[:], gs[:, H1 : 2 * H1], adx_p[:])
                es = wp.tile([P, H1], f32, tag="es")
                nc.vector.tensor_scalar_mul(es[:], e[:], 0.2)
                el = wp.tile([P, H1], f32, tag="el")
                nc.vector.tensor_tensor(out=el[:], in0=e[:], in1=es[:], op=MAX)
                pe = wp.tile([P, H1], f32, tag="pe")
                nc.scalar.activation(pe[:], el[:], Exp)
                rhs_t = wp.tile([P, D1 + H1], f32, tag="rhs_t")
                for h in range(H1):
                    sl = slice(h * C, (h + 1) * C)
                    gsl = slice(2 * H1 + h * C, 2 * H1 + (h + 1) * C)
                    if h % 2 == 0:
                        nc.scalar.activation(
                            rhs_t[:, sl], gs[:, gsl], Copy,
                            scale=pe[:, h : h + 1],
                        )
                    else:
                        nc.vector.tensor_scalar_mul(
                            rhs_t[:, sl], gs[:, gsl], pe[:, h : h + 1]
                        )
                nc.vector.tensor_copy(rhs_t[:, D1:], pe[:])
                nc.tensor.matmul(
                    out=agg1[:], lhsT=em[:], rhs=rhs_t[:],
                    start=(t == 0), stop=(t == T1 - 1),
                )

            # ---------------- layer-1 finalize: h1r = relu(num/den + b1) ---
            den1 = fp.tile([P, H1], f32)
            nc.vector.tensor_scalar_add(den1[:], agg1[:, D1:], 1e-16)
            rec1 = fp.tile([P, H1], f32)
            nc.vector.reciprocal(rec1[:], den1[:])
            h1t = fp.tile([P, D1], f32)
            for h in range(H1):
                sl = slice(h * C, (h + 1) * C)
                nc.scalar.activation(
                    h1t[:, sl], agg1[:, sl], Copy, scale=rec1[:, h : h + 1]
                )
            h1b = fp.tile([P, D1], f32)
            nc.vector.tensor_add(h1b[:], h1t[:], fsl("b1bc"))
            h1r = fp.tile([P, D1], f32)
            nc.scalar.activation(h1r[:], h1b[:], Relu)

            # ---------------- layer-2 projection: G2 = [ad2|as2|h2p] -------
            pg2 = ap_.tile([P, G2W], f32, tag="pg2")
            for k in range(KCH):
                ptr = pp.tile([P, P], f32, tag="mm")
                nc.tensor.transpose(
                    out=ptr[:], in_=h1r[:, k * P : (k + 1) * P], identity=ident[:]
                )
                h1rT_k = wp.tile([P, P], f32, tag=f"h1rTk{k}")
                nc.vector.tensor_copy(h1rT_k[:], ptr[:])
                pwa2 = pp.tile([P, 2], f32, tag="mm")
                nc.tensor.matmul(
                    out=pwa2[:], lhsT=fsl("w2T", C)[:, k * P : (k + 1) * P],
                    rhs=fsl("a2", C), start=True, stop=True,
                )
                rhs2_k = wp.tile([P, G2W], f32, tag=f"rhs2k{k}")
                nc.vector.tensor_copy(rhs2_k[:, :2], pwa2[:])
                nc.scalar.copy(rhs2_k[:, 2:], fsl(f"w2_{k}"))
                nc.tensor.matmul(
                    out=pg2[:], lhsT=h1rT_k[:], rhs=rhs2_k[:],
                    start=(k == 0), stop=(k == KCH - 1),
                )
            g2sb = fp.tile([P, G2W], f32)
            nc.scalar.copy(g2sb[:], pg2[:])
            nc.sync.dma_start(g2_d[:], g2sb[:])
            ad2_slots = g2sb[:, 0:1]

            # ---------------- layer-2 edge aggregation (ball only) ---------
            agg2 = ap_.tile([P, C + 1], f32, tag="agg2")
            drel2 = fsl("dstrel2")
            src2 = isl("src2")
            for t in range(T2):
                gs2 = wp.tile([P, G2W], f32, tag="gs2")
                nc.gpsimd.indirect_dma_start(
                    out=gs2[:], out_offset=None, in_=g2_d[:, :],
                    in_offset=AX0(src2[:, t : t + 1]),
                )
                em2 = wp.tile([P, P], f32, tag="em2")
                nc.vector.tensor_scalar(
                    em2[:], iota_f[:], drel2[:, t : t + 1], None, EQ
                )
                emt2_p = pp.tile([P, P], f32, tag="mm")
                nc.tensor.transpose(out=emt2_p[:], in_=em2[:], identity=ident[:])
                emt2 = wp.tile([P, P], f32, tag="emt2")
                nc.scalar.copy(emt2[:], emt2_p[:])
                adx2_p = pp.tile([P, 1], f32, tag="mm")
                nc.tensor.matmul(
                    out=adx2_p[:], lhsT=emt2[:], rhs=ad2_slots,
                    start=True, stop=True,
                )
                e2 = wp.tile([P, 1], f32, tag="e2")
                nc.vector.tensor_add(e2[:], gs2[:, 1:2], adx2_p[:])
                es2 = wp.tile([P, 1], f32, tag="es2")
                nc.vector.tensor_scalar_mul(es2[:], e2[:], 0.2)
                el2 = wp.tile([P, 1], f32, tag="el2")
                nc.vector.tensor_tensor(out=el2[:], in0=e2[:], in1=es2[:], op=MAX)
                pe2 = wp.tile([P, 1], f32, tag="pe2")
                nc.scalar.activation(pe2[:], el2[:], Exp)
                rhs2t = wp.tile([P, C + 1], f32, tag="rhs2t")
                nc.scalar.activation(
                    rhs2t[:, :C], gs2[:, 2:], Copy, scale=pe2[:, 0:1]
                )
                nc.vector.tensor_copy(rhs2t[:, C:], pe2[:])
                nc.tensor.matmul(
                    out=agg2[:], lhsT=em2[:], rhs=rhs2t[:],
                    start=(t == 0), stop=(t == T2 - 1),
                )

            # ---------------- ball finalize + MLP --------------------------
            den2 = fp.tile([1, 1], f32)
            nc.vector.tensor_scalar_add(den2[:], agg2[0:1, C : C + 1], 1e-16)
            rec2 = fp.tile([1, 1], f32)
            nc.vector.reciprocal(rec2[:], den2[:])
            bf = fp.tile([1, C], f32)
            nc.scalar.activation(bf[:], agg2[0:1, :C], Copy, scale=rec2[:, 0:1])
            bfb = fp.tile([1, C], f32)
            nc.vector.tensor_add(bfb[:], bf[:], fsl("b2row", 1))
            bfr = fp.tile([1, C], f32)
            nc.scalar.activation(bfr[:], bfb[:], Relu)

            ptb = pp.tile([C, 1], f32, tag="mm")
            nc.tensor.transpose(out=ptb[:], in_=bfr[:], identity=ident[0:1, 0:1])
            bcol = fp.tile([C, 1], f32)
            nc.vector.tensor_copy(bcol[:], ptb[:])

            pz = pp.tile([C // 2, 1], f32, tag="mm")
            nc.tensor.matmul(out=pz[:], lhsT=fsl("fc1w", C), rhs=bcol[:],
                             start=True, stop=True)
            zb = fp.tile([C // 2, 1], f32)
            nc.vector.tensor_add(zb[:], pz[:], fsl("fc1b", C // 2))
            zr = fp.tile([C // 2, 1], f32)
            nc.scalar.activation(zr[:], zb[:], Relu)

            po = pp.tile([2, 1], f32, tag="mm")
            nc.tensor.matmul(out=po[:], lhsT=fsl("fc2w", C // 2), rhs=zr[:],
                             start=True, stop=True)
            osb = fp.tile([2, 1], f32)
            nc.vector.tensor_add(osb[:], po[:], fsl("fc2b", 2))
            nc.sync.dma_start(out_d[:], osb[:])

    nc.compile()
    return nc


def kernel(**inputs):
    from concourse.bass_utils import run_bass_kernel_spmd

    feed, dims = _host_preprocess(inputs)
    key = (dims["m1p"], dims["m2"], dims["T1"], dims["T2"])
    if key not in _CACHE:
        _CACHE[key] = _build(dims)
    nc = _CACHE[key]

    n_cores = 8
    in_maps = [dict(feed) for _ in range(n_cores)]
    res = run_bass_kernel_spmd(nc, in_maps, core_ids=list(range(n_cores)))
    out = np.asarray(res.results[0]["out"], dtype=np.float32).reshape(2)
    return out


# revision 8
# speedup vs baseline: 1.3159x; 1.0417x over previous
"""Trainium2 Bass kernel for nn_BallPredictorGNN.

The reference model is a 2-layer GAT over (N=20000, E=640000) followed by an
MLP applied to the LAST node only ("ball") — the output is a single [2] vector.
Only the ball's 2-hop dependency cone matters:

  layer 2 aggregates at the ball node only            (~25 in-edges)
  layer 1 aggregates at the ball's in-neighbours S2   (~25 nodes, ~800 edges)
  x @ W1 is needed for the sources of those edges S1  (~800 nodes)

Host side (pure data routing): extract the cone, sort edges by destination,
build padded index tables, pack small operands.  Device side (all FLOPs):
dense projections via TensorE; per-128-edge source-row gathers via indirect
DMA from a DRAM table; segment-softmax-aggregation via one-hot matmuls with
fused numerator/denominator accumulation in PSUM
(alpha = exp(e)/sum exp(e) folded as  out = (sum exp(e)*h_src) / sum exp(e)).
The per-edge a_dst[dst] term is expanded on-chip as Ematᵀ.T @ ad_slots, where
all Ematᵀ slices are built in two matmuls (row-replicate) + is_equal compares.
Layer 2 runs entirely on-chip: its "gather" is a one-hot matmul against the
SBUF-resident G2 table (no DRAM round-trip).

The same program is replicated SPMD on all 8 NeuronCores (the cone is tiny, so
replication beats sharding + collectives); core 0's output is returned.
"""

import numpy as np

P = 128
_CACHE = {}


def _ceil(a, b):
    return -(-a // b)


def _pad_rows(a, n, fill=0):
    out = np.full((n,) + a.shape[1:], fill, a.dtype)
    out[: len(a)] = a
    return out


class _Packer:
    """Pack many small [p, w] operands into one [128, W] array, column-wise."""

    def __init__(self):
        self.cols = []
        self.pos = 0
        self.slots = {}

    def add(self, name, arr):
        p, w = arr.shape
        full = np.zeros((P, w), arr.dtype)
        full[:p] = arr
        self.cols.append(full)
        self.slots[name] = (self.pos, self.pos + w)
        self.pos += w

    def finish(self):
        return np.ascontiguousarray(np.concatenate(self.cols, axis=1))


def _host_preprocess(inputs):
    x = np.ascontiguousarray(np.asarray(inputs["x"], dtype=np.float32))
    ei = np.asarray(inputs["edge_index"]).astype(np.int64)
    N = x.shape[0]
    F = x.shape[1]
    ball = N - 1
    src, dst = ei[0], ei[1]

    # ---- layer-2 cone: edges into the ball (+ self loop) --------------------
    sel2 = dst == ball
    e2s = np.concatenate([src[sel2], [ball]])
    uniq = np.unique(e2s)
    S2 = np.concatenate([[ball], uniq[uniq != ball]]).astype(np.int64)
    m2 = len(S2)
    assert m2 <= 127, f"ball in-neighbourhood too large for one dst block: {m2}"

    # ---- layer-1 cone: edges into S2 (+ self loops for S2) ------------------
    in_S2 = np.zeros(N, dtype=bool)
    in_S2[S2] = True
    sel1 = in_S2[dst]
    e1s = np.concatenate([src[sel1], S2])
    e1d = np.concatenate([dst[sel1], S2])
    uniq1 = np.unique(e1s)
    rest = uniq1[~in_S2[uniq1]]
    S1 = np.concatenate([S2, rest])  # S2 is a prefix of S1
    m1 = len(S1)
    m1p = _ceil(m1, P) * P

    loc1 = np.full(N, -1, dtype=np.int64)
    loc1[S1] = np.arange(m1)
    s_loc = loc1[e1s]
    d_loc = loc1[e1d]  # in [0, m2)
    order = np.argsort(d_loc, kind="stable")
    s_loc, d_loc = s_loc[order], d_loc[order]
    n1 = len(s_loc)
    T1 = _ceil(n1, P)
    n1p = T1 * P

    # per-tile column layout [P, T]: element (p, t) = edge t*P + p
    def cols(a, n_pad, fill, dt):
        return np.ascontiguousarray(
            _pad_rows(a.astype(dt), n_pad, fill).reshape(-1, P).T
        )

    def row(a, n_pad, fill, dt):
        return _pad_rows(a.astype(dt), n_pad, fill)[None, :]

    s2_loc = loc1[e2s]  # all < m2
    n2 = len(s2_loc)
    T2 = _ceil(n2, P)
    assert T2 == 1, f"layer-2 edge count exceeds one tile: {n2}"
    n2p = T2 * P

    pki = _Packer()
    pki.add("src1", cols(s_loc, n1p, 0, np.int32))

    # ---- dense operands -----------------------------------------------------
    xT = np.ascontiguousarray(_pad_rows(x[S1], m1p).T)  # [F, m1p]

    W1 = np.asarray(inputs["W1"], np.float32)  # [F, 4*64]
    a_src1 = np.asarray(inputs["a_src1"], np.float32)  # [4, 64]
    a_dst1 = np.asarray(inputs["a_dst1"], np.float32)
    H1, C = a_src1.shape
    D1 = H1 * C
    ablk = np.zeros((D1, 2 * H1), np.float32)  # [256, 8] = [Ad | As]
    for h in range(H1):
        ablk[h * C : (h + 1) * C, h] = a_dst1[h]
        ablk[h * C : (h + 1) * C, H1 + h] = a_src1[h]

    W2 = np.asarray(inputs["W2"], np.float32)  # [256, 64]
    a2 = np.stack(
        [np.asarray(inputs["a_dst2"], np.float32)[0],
         np.asarray(inputs["a_src2"], np.float32)[0]],
        axis=1,
    )  # [64, 2] = [a_dst | a_src]

    pkf = _Packer()
    pkf.add("w1", W1)
    W1T = np.ascontiguousarray(W1.T)
    for k in range(D1 // P):
        pkf.add(f"w1T{k}", W1T[k * P : (k + 1) * P])
        pkf.add(f"ablk{k}", ablk[k * P : (k + 1) * P])
        pkf.add(f"w2_{k}", W2[k * P : (k + 1) * P])
    pkf.add("b1bc", np.broadcast_to(np.asarray(inputs["b1"], np.float32), (P, D1)))
    pkf.add("w2T", np.ascontiguousarray(W2.T))
    pkf.add("a2", a2)
    pkf.add("b2col", np.asarray(inputs["b2"], np.float32)[:, None])
    pkf.add("fc1w", np.ascontiguousarray(np.asarray(inputs["fc1_w"], np.float32)))
    pkf.add("fc1b", np.asarray(inputs["fc1_b"], np.float32)[:, None])
    pkf.add("fc2w", np.ascontiguousarray(np.asarray(inputs["fc2_w"], np.float32)))
    pkf.add("fc2b", np.asarray(inputs["fc2_b"], np.float32)[:, None])
    pkf.add("dstrel1", cols(d_loc, n1p, m2, np.float32))  # pad -> garbage slot
    pkf.add("dstrel2", cols(np.zeros(n2, np.int64), n2p, 1, np.float32))
    pkf.add("dstrel1r", row(d_loc, n1p, m2, np.float32))
    pkf.add("l2rows", np.concatenate(
        [row(s2_loc, n2p, 0, np.float32), row(np.zeros(n2), n2p, 1, np.float32)],
        axis=1,
    ))

    feed = {"xT": xT, "packf": pkf.finish(), "packi": pki.finish()}
    dims = dict(
        F=F, H1=H1, C=C, m1p=m1p, m2=m2, T1=T1, T2=T2,
        slots_f=tuple(sorted(pkf.slots.items())),
        slots_i=tuple(sorted(pki.slots.items())),
    )
    return feed, dims


def _build(dims):
    from concourse import bacc, bass, mybir, tile
    from concourse.masks import make_identity

    F = dims["F"]          # 128 input features
    H1 = dims["H1"]        # 4 heads, layer 1
    C = dims["C"]          # 64 channels per head
    D1 = H1 * C            # 256
    G1W = 2 * H1 + D1      # 264 = [ad(4) | as(4) | h(256)]
    G2W = 2 + C            # 66  = [ad2 | as2 | h2p]
    m1p, T1 = dims["m1p"], dims["T1"]
    n1p = T1 * P
    NCH1 = m1p // P
    KCH = D1 // P          # 2 contraction chunks over 256
    slots_f = dict(dims["slots_f"])
    slots_i = dict(dims["slots_i"])
    WF = max(b for _, b in slots_f.values())
    WI = max(b for _, b in slots_i.values())
    f32 = mybir.dt.float32
    i32 = mybir.dt.int32
    AX0 = lambda ap: bass.IndirectOffsetOnAxis(ap=ap, axis=0)

    nc = bacc.Bacc("TRN2", target_bir_lowering=False, debug=False)

    xT_d = nc.declare_dram_parameter("xT", [F, m1p], f32, isOutput=False)
    pf_d = nc.declare_dram_parameter("packf", [P, WF], f32, isOutput=False)
    pi_d = nc.declare_dram_parameter("packi", [P, WI], i32, isOutput=False)
    out_d = nc.declare_dram_parameter("out", [2, 1], f32, isOutput=True)

    g1_d = nc.dram_tensor("g1_tab", [m1p, G1W], f32)

    EQ = mybir.AluOpType.is_equal
    MAX = mybir.AluOpType.max
    MUL = mybir.AluOpType.mult
    Copy = mybir.ActivationFunctionType.Copy
    Exp = mybir.ActivationFunctionType.Exp
    Relu = mybir.ActivationFunctionType.Relu

    with tile.TileContext(nc) as tc:
        with (
            tc.tile_pool(name="const", bufs=1) as cp,
            tc.tile_pool(name="work", bufs=3) as wp,
            tc.tile_pool(name="fin", bufs=1) as fp,
            tc.tile_pool(name="psum", bufs=2, space="PSUM") as pp,
            tc.tile_pool(name="acc", bufs=1, space="PSUM") as ap_,
        ):
            # ---------------- constants / inputs into SBUF ----------------
            pk = cp.tile([P, WF], f32)
            nc.sync.dma_start(pk[:], pf_d[:])
            pki_s = cp.tile([P, WI], i32)
            nc.sync.dma_start(pki_s[:], pi_d[:])
            xT_s = cp.tile([F, m1p], f32)
            nc.sync.dma_start(xT_s[:], xT_d[:])

            def fsl(name, rows=P):
                a, b = slots_f[name]
                return pk[:rows, a:b]

            def isl(name):
                a, b = slots_i[name]
                return pki_s[:, a:b]

            ident = cp.tile([P, P], f32)
            make_identity(nc, ident[:])
            iota_f = cp.tile([P, P], f32)
            nc.gpsimd.iota(
                iota_f[:], pattern=[[1, P]], base=0, channel_multiplier=0,
                allow_small_or_imprecise_dtypes=True,
            )
            iota_c = cp.tile([P, 1], f32)
            nc.gpsimd.iota(
                iota_c[:], pattern=[[0, 1]], base=0, channel_multiplier=1,
                allow_small_or_imprecise_dtypes=True,
            )
            ones1 = cp.tile([1, P], f32)
            nc.gpsimd.memset(ones1[:], 1.0)

            # ---------------- W1 @ [Ad | As]  (K = 256, 2 chunks) ----------
            pwa = pp.tile([F, 2 * H1], f32, tag="mm")
            for k in range(KCH):
                nc.tensor.matmul(
                    out=pwa[:], lhsT=fsl(f"w1T{k}"), rhs=fsl(f"ablk{k}"),
                    start=(k == 0), stop=(k == KCH - 1),
                )
            rhs1 = cp.tile([F, G1W], f32)
            nc.vector.tensor_copy(rhs1[:, : 2 * H1], pwa[:])
            nc.scalar.copy(rhs1[:, 2 * H1 :], fsl("w1"))

            # ---- all Ematᵀ slices: replicate dstrel1 row, compare to iota --
            emt_all = cp.tile([P, n1p], f32)
            dr1r = fsl("dstrel1r", 1)
            for c0 in range(0, n1p, 512):
                cw = min(512, n1p - c0)
                prep = pp.tile([P, 512], f32, tag="pg")
                nc.tensor.matmul(
                    out=prep[:, :cw], lhsT=ones1[:], rhs=dr1r[:, c0 : c0 + cw],
                    start=True, stop=True,
                )
                nc.vector.tensor_scalar(
                    emt_all[:, c0 : c0 + cw], prep[:, :cw], iota_c[:, 0:1],
                    None, EQ,
                )

            # -------- G1 rows: [ad | as | h] = x @ [W1Ad | W1As | W1] ------
            gall = fp.tile([P, NCH1 * G1W], f32)
            for ci in range(NCH1):
                pg = pp.tile([P, 512], f32, tag="pg")
                nc.tensor.matmul(
                    out=pg[:, :G1W], lhsT=xT_s[:, ci * P : (ci + 1) * P],
                    rhs=rhs1[:], start=True, stop=True,
                )
                if ci % 2:
                    nc.scalar.copy(gall[:, ci * G1W : (ci + 1) * G1W], pg[:, :G1W])
                else:
                    nc.vector.tensor_copy(
                        gall[:, ci * G1W : (ci + 1) * G1W], pg[:, :G1W]
                    )
            gall3 = gall[:].rearrange("p (c w) -> p c w", w=G1W)
            nc.sync.dma_start(
                g1_d[:].rearrange("(c p) w -> p c w", p=P), gall3
            )
            # a_dst values for the destination slots: S2 is a prefix of S1,
            # so slot s lives in pre-phase chunk 0, row s, cols 0:H1.
            ad_slots = gall[:, 0:H1]

            # ---------------- layer-1 edge aggregation ---------------------
            agg1 = ap_.tile([P, D1 + H1], f32, tag="agg1")
            drel1 = fsl("dstrel1")
            src1 = isl("src1")
            for t in range(T1):
                gs = wp.tile([P, G1W], f32, tag="gs")
                nc.gpsimd.indirect_dma_start(
                    out=gs[:], out_offset=None, in_=g1_d[:, :],
                    in_offset=AX0(src1[:, t : t + 1]),
                )
                adx_p = pp.tile([P, H1], f32, tag="mm")
                nc.tensor.matmul(
                    out=adx_p[:], lhsT=emt_all[:, t * P : (t + 1) * P],
                    rhs=ad_slots, start=True, stop=True,
                )
                e = wp.tile([P, H1], f32, tag="e")
                nc.vector.tensor_add(e[:], gs[:, H1 : 2 * H1], adx_p[:])
                es = wp.tile([P, H1], f32, tag="es")
                nc.vector.tensor_scalar_mul(es[:], e[:], 0.2)
                el = wp.tile([P, H1], f32, tag="el")
                nc.vector.tensor_tensor(out=el[:], in0=e[:], in1=es[:], op=MAX)
                pe = wp.tile([P, H1], f32, tag="pe")
                nc.scalar.activation(pe[:], el[:], Exp)
                rhs_t = wp.tile([P, D1 + H1], f32, tag="rhs_t")
                nc.vector.tensor_tensor(
                    out=rhs_t[:, :D1].rearrange("p (h c) -> p h c", c=C),
                    in0=gs[:, 2 * H1 :].rearrange("p (h c) -> p h c", c=C),
                    in1=pe[:].rearrange("p (h o) -> p h o", o=1).to_broadcast(
                        [P, H1, C]
                    ),
                    op=MUL,
                )
                nc.scalar.activation(rhs_t[:, D1:], pe[:], Copy)
                em = wp.tile([P, P], f32, tag="em")
                nc.vector.tensor_scalar(
                    em[:], iota_f[:], drel1[:, t : t + 1], None, EQ
                )
                nc.tensor.matmul(
                    out=agg1[:], lhsT=em[:], rhs=rhs_t[:],
                    start=(t == 0), stop=(t == T1 - 1),
                )

            # ---------------- layer-1 finalize: h1r = relu(num/den + b1) ---
            den1 = fp.tile([P, H1], f32)
            nc.vector.tensor_scalar_add(den1[:], agg1[:, D1:], 1e-16)
            rec1 = fp.tile([P, H1], f32)
            nc.vector.reciprocal(rec1[:], den1[:])
            h1t = fp.tile([P, D1], f32)
            nc.vector.tensor_tensor(
                out=h1t[:].rearrange("p (h c) -> p h c", c=C),
                in0=agg1[:, :D1].rearrange("p (h c) -> p h c", c=C),
                in1=rec1[:].rearrange("p (h o) -> p h o", o=1).to_broadcast(
                    [P, H1, C]
                ),
                op=MUL,
            )
            h1b = fp.tile([P, D1], f32)
            nc.vector.tensor_add(h1b[:], h1t[:], fsl("b1bc"))
            h1r = fp.tile([P, D1], f32)
            nc.scalar.activation(h1r[:], h1b[:], Relu)

            # ---------------- layer-2 projection: G2 = [ad2|as2|h2p] -------
            pg2 = ap_.tile([P, G2W], f32, tag="pg2")
            for k in range(KCH):
                ptr = pp.tile([P, P], f32, tag="mm")
                nc.tensor.transpose(
                    out=ptr[:], in_=h1r[:, k * P : (k + 1) * P], identity=ident[:]
                )
                h1rT_k = wp.tile([P, P], f32, tag=f"h1rTk{k}")
                nc.vector.tensor_copy(h1rT_k[:], ptr[:])
                pwa2 = pp.tile([P, 2], f32, tag="mm")
                nc.tensor.matmul(
                    out=pwa2[:], lhsT=fsl("w2T", C)[:, k * P : (k + 1) * P],
                    rhs=fsl("a2", C), start=True, stop=True,
                )
                rhs2_k = wp.tile([P, G2W], f32, tag=f"rhs2k{k}")
                nc.vector.tensor_copy(rhs2_k[:, :2], pwa2[:])
                nc.scalar.copy(rhs2_k[:, 2:], fsl(f"w2_{k}"))
                nc.tensor.matmul(
                    out=pg2[:], lhsT=h1rT_k[:], rhs=rhs2_k[:],
                    start=(k == 0), stop=(k == KCH - 1),
                )
            g2sb = fp.tile([P, G2W], f32)
            nc.scalar.copy(g2sb[:], pg2[:])

            # ------- layer-2 edge aggregation: fully on-chip (ball only) ---
            # one-hot "gather" + adᵀ expansion from replicated index rows
            prep2 = pp.tile([P, 2 * P], f32, tag="mm")
            nc.tensor.matmul(
                out=prep2[:], lhsT=ones1[:], rhs=fsl("l2rows", 1),
                start=True, stop=True,
            )
            st2 = fp.tile([P, P], f32)
            nc.vector.tensor_scalar(st2[:], prep2[:, :P], iota_c[:, 0:1], None, EQ)
            em2t = fp.tile([P, P], f32)
            nc.vector.tensor_scalar(em2t[:], prep2[:, P:], iota_c[:, 0:1], None, EQ)

            gs2_p = pp.tile([P, G2W], f32, tag="mm")
            nc.tensor.matmul(out=gs2_p[:], lhsT=st2[:], rhs=g2sb[:],
                             start=True, stop=False, skip_group_check=True)
            # accumulate the a_dst[ball] expansion straight onto the as2
            # column: gs2_p[:, 1] becomes e2 = as2[src] + ad2[dst]
            nc.tensor.matmul(out=gs2_p[:, 1:2], lhsT=em2t[:], rhs=g2sb[:, 0:1],
                             start=False, stop=True, skip_group_check=True)
            es2 = fp.tile([P, 1], f32)
            nc.vector.tensor_scalar_mul(es2[:], gs2_p[:, 1:2], 0.2)
            el2 = fp.tile([P, 1], f32)
            nc.vector.tensor_tensor(
                out=el2[:], in0=gs2_p[:, 1:2], in1=es2[:], op=MAX
            )
            pe2 = fp.tile([P, 1], f32)
            nc.scalar.activation(pe2[:], el2[:], Exp)
            rhs2t = fp.tile([P, C + 1], f32)
            nc.vector.tensor_tensor(
                out=rhs2t[:, :C], in0=gs2_p[:, 2:],
                in1=pe2[:].to_broadcast([P, C]), op=MUL,
            )
            nc.scalar.activation(rhs2t[:, C:], pe2[:], Copy)
            em2 = fp.tile([P, P], f32)
            nc.vector.tensor_scalar(
                em2[:], iota_f[:], fsl("dstrel2")[:, 0:1], None, EQ
            )
            agg2 = ap_.tile([P, C + 1], f32, tag="agg2")
            nc.tensor.matmul(out=agg2[:], lhsT=em2[:], rhs=rhs2t[:],
                             start=True, stop=True)

            # ---------------- ball finalize + MLP --------------------------
            den2 = fp.tile([1, 1], f32)
            nc.vector.tensor_scalar_add(den2[:], agg2[0:1, C : C + 1], 1e-16)
            rec2 = fp.tile([1, 1], f32)
            nc.vector.reciprocal(rec2[:], den2[:])
            bf = fp.tile([1, C], f32)
            nc.scalar.activation(bf[:], agg2[0:1, :C], Copy, scale=rec2[:, 0:1])
            ptb = pp.tile([C, 1], f32, tag="mm")
            nc.tensor.transpose(out=ptb[:], in_=bf[:], identity=ident[0:1, 0:1])
            bfr = fp.tile([C, 1], f32)
            nc.scalar.activation(bfr[:], ptb[:], Relu, bias=fsl("b2col", C))

            pz = pp.tile([C // 2, 1], f32, tag="mm")
            nc.tensor.matmul(out=pz[:], lhsT=fsl("fc1w", C), rhs=bfr[:],
                             start=True, stop=True)
            zr = fp.tile([C // 2, 1], f32)
            nc.scalar.activation(zr[:], pz[:], Relu, bias=fsl("fc1b", C // 2))

            po = pp.tile([2, 1], f32, tag="mm")
            nc.tensor.matmul(out=po[:], lhsT=fsl("fc2w", C // 2), rhs=zr[:],
                             start=True, stop=True)
            osb = fp.tile([2, 1], f32)
            nc.vector.tensor_add(osb[:], po[:], fsl("fc2b", 2))
            nc.sync.dma_start(out_d[:], osb[:])

    nc.compile()
    return nc


def kernel(**inputs):
    from concourse.bass_utils import run_bass_kernel_spmd

    feed, dims = _host_preprocess(inputs)
    key = (dims["m1p"], dims["m2"], dims["T1"], dims["T2"])
    if key not in _CACHE:
        _CACHE[key] = _build(dims)
    nc = _CACHE[key]

    n_cores = 8
    in_maps = [dict(feed) for _ in range(n_cores)]
    res = run_bass_kernel_spmd(nc, in_maps, core_ids=list(range(n_cores)))
    out = np.asarray(res.results[0]["out"], dtype=np.float32).reshape(2)
    return out


# revision 10
# speedup vs baseline: 1.6612x; 1.2624x over previous
"""Trainium2 Bass kernel for nn_BallPredictorGNN.

The reference model is a 2-layer GAT over (N=20000, E=640000) followed by an
MLP applied to the LAST node only ("ball") — the output is a single [2] vector.
Only the ball's 2-hop dependency cone matters:

  layer 2 aggregates at the ball node only            (~25 in-edges)
  layer 1 aggregates at the ball's in-neighbours S2   (~25 nodes, ~800 edges)
  x @ W1 is needed for the sources of those edges S1  (~800 nodes)

Host side (pure data routing): extract the cone, sort edges by destination,
order source nodes by first use so the gather pipeline can start before the
whole projection table is written, build padded index tables, pack small
operands.  Device side (all FLOPs): dense projections via TensorE;
per-128-edge source-row gathers via indirect DMA from a DRAM table;
segment-softmax-aggregation via one-hot matmuls (bf16) with fused numerator /
denominator accumulation in PSUM
(alpha = exp(e)/sum exp(e) folded as  out = (sum exp(e)*h_src) / sum exp(e)).
The per-edge a_dst[dst] term is expanded on-chip as Ematᵀ.T @ ad_slots with
Ematᵀ built from a host-replicated index row.  Layer 2 runs entirely on-chip:
its "gather" is a one-hot matmul against the SBUF-resident G2 table.

The same program is replicated SPMD on all 8 NeuronCores (the cone is tiny, so
replication beats sharding + collectives); core 0's output is returned.
"""

import numpy as np

P = 128
_CACHE = {}


def _ceil(a, b):
    return -(-a // b)


def _pad_rows(a, n, fill=0):
    out = np.full((n,) + a.shape[1:], fill, a.dtype)
    out[: len(a)] = a
    return out


class _Packer:
    """Pack many small [p, w] operands into one [128, W] array, column-wise."""

    def __init__(self):
        self.cols = []
        self.pos = 0
        self.slots = {}

    def add(self, name, arr):
        p, w = arr.shape
        full = np.zeros((P, w), arr.dtype)
        full[:p] = arr
        self.cols.append(full)
        self.slots[name] = (self.pos, self.pos + w)
        self.pos += w

    def finish(self):
        return np.ascontiguousarray(np.concatenate(self.cols, axis=1))


def _host_preprocess(inputs):
    x = np.ascontiguousarray(np.asarray(inputs["x"], dtype=np.float32))
    ei = np.asarray(inputs["edge_index"]).astype(np.int64)
    N = x.shape[0]
    F = x.shape[1]
    ball = N - 1
    src, dst = ei[0], ei[1]

    # ---- layer-2 cone: edges into the ball (+ self loop) --------------------
    sel2 = dst == ball
    e2s = np.concatenate([src[sel2], [ball]])
    uniq = np.unique(e2s)
    S2 = np.concatenate([[ball], uniq[uniq != ball]]).astype(np.int64)
    m2 = len(S2)
    assert m2 <= 127, f"ball in-neighbourhood too large for one dst block: {m2}"

    # ---- layer-1 cone: edges into S2 (+ self loops for S2) ------------------
    in_S2 = np.zeros(N, dtype=bool)
    in_S2[S2] = True
    sel1 = in_S2[dst]
    e1s = np.concatenate([src[sel1], S2])
    e1d = np.concatenate([dst[sel1], S2])

    # sort edges by destination slot (S2 order)
    loc2 = np.full(N, -1, dtype=np.int64)
    loc2[S2] = np.arange(m2)
    d_slot = loc2[e1d]
    order = np.argsort(d_slot, kind="stable")
    e1s, d_slot = e1s[order], d_slot[order]
    n1 = len(e1s)
    T1 = _ceil(n1, P)
    n1p = T1 * P

    # order S1 = [S2 | rest-by-first-use-in-edge-stream] so that gather tile t
    # only touches low G1 rows -> gathers overlap the projection pipeline
    seen = in_S2.copy()
    rest = []
    for s in e1s:
        if not seen[s]:
            seen[s] = True
            rest.append(s)
    S1 = np.concatenate([S2, np.asarray(rest, dtype=np.int64)])
    m1 = len(S1)
    m1p = _ceil(m1, P) * P
    loc1 = np.full(N, -1, dtype=np.int64)
    loc1[S1] = np.arange(m1)
    s_loc = loc1[e1s]

    # rows of G1 needed by each gather tile (for sliced dependency ranges)
    smax = np.maximum.accumulate(
        np.max(_pad_rows(s_loc, n1p).reshape(T1, P), axis=1)
    )
    rows_needed = np.minimum(
        (_ceil(int(m2), P) + 0) * 0 + (smax // P + 1) * P, m1p
    ).astype(int)

    # per-tile column layout [P, T]: element (p, t) = edge t*P + p
    def cols(a, n_pad, fill, dt):
        return np.ascontiguousarray(
            _pad_rows(a.astype(dt), n_pad, fill).reshape(-1, P).T
        )

    def row(a, n_pad, fill, dt):
        return _pad_rows(a.astype(dt), n_pad, fill)[None, :]

    s2_loc = loc1[e2s]  # all < m2
    n2 = len(s2_loc)
    T2 = _ceil(n2, P)
    assert T2 == 1, f"layer-2 edge count exceeds one tile: {n2}"
    n2p = T2 * P

    pki = _Packer()
    pki.add("src1", cols(s_loc, n1p, 0, np.int32))

    # ---- dense operands -----------------------------------------------------
    xT = np.ascontiguousarray(_pad_rows(x[S1], m1p).T)  # [F, m1p]

    W1 = np.asarray(inputs["W1"], np.float32)  # [F, 4*64]
    a_src1 = np.asarray(inputs["a_src1"], np.float32)  # [4, 64]
    a_dst1 = np.asarray(inputs["a_dst1"], np.float32)
    H1, C = a_src1.shape
    D1 = H1 * C
    ablk = np.zeros((D1, 2 * H1), np.float32)  # [256, 8] = [Ad | As]
    for h in range(H1):
        ablk[h * C : (h + 1) * C, h] = a_dst1[h]
        ablk[h * C : (h + 1) * C, H1 + h] = a_src1[h]

    W2 = np.asarray(inputs["W2"], np.float32)  # [256, 64]
    a2 = np.stack(
        [np.asarray(inputs["a_dst2"], np.float32)[0],
         np.asarray(inputs["a_src2"], np.float32)[0]],
        axis=1,
    )  # [64, 2] = [a_dst | a_src]

    pkf = _Packer()
    pkf.add("w1", W1)
    W1T = np.ascontiguousarray(W1.T)
    for k in range(D1 // P):
        pkf.add(f"w1T{k}", W1T[k * P : (k + 1) * P])
        pkf.add(f"ablk{k}", ablk[k * P : (k + 1) * P])
        pkf.add(f"w2_{k}", W2[k * P : (k + 1) * P])
    pkf.add("b1bc", np.broadcast_to(np.asarray(inputs["b1"], np.float32), (P, D1)))
    pkf.add("w2T", np.ascontiguousarray(W2.T))
    pkf.add("a2", a2)
    pkf.add("b2col", np.asarray(inputs["b2"], np.float32)[:, None])
    pkf.add("fc1w", np.ascontiguousarray(np.asarray(inputs["fc1_w"], np.float32)))
    pkf.add("fc1b", np.asarray(inputs["fc1_b"], np.float32)[:, None])
    pkf.add("fc2w", np.ascontiguousarray(np.asarray(inputs["fc2_w"], np.float32)))
    pkf.add("fc2b", np.asarray(inputs["fc2_b"], np.float32)[:, None])
    pkf.add("dstrel1", cols(d_slot, n1p, m2, np.float32))  # pad -> garbage slot
    pkf.add("dstrel2", cols(np.zeros(n2, np.int64), n2p, 1, np.float32))

    pkr = _Packer()
    pkr.add("drep1", np.broadcast_to(
        row(d_slot, n1p, m2, np.float32), (P, n1p)))
    pkr.add("l2rep", np.broadcast_to(np.concatenate(
        [row(s2_loc, n2p, 0, np.float32),
         row(np.zeros(n2), n2p, 1, np.float32)], axis=1), (P, 2 * n2p)))

    feed = {
        "xT": xT, "packf": pkf.finish(), "packi": pki.finish(),
        "packr": pkr.finish(),
    }
    dims = dict(
        F=F, H1=H1, C=C, m1p=m1p, m2=m2, T1=T1, T2=T2,
        rows_needed=tuple(int(r) for r in rows_needed),
        slots_f=tuple(sorted(pkf.slots.items())),
        slots_i=tuple(sorted(pki.slots.items())),
        slots_r=tuple(sorted(pkr.slots.items())),
    )
    return feed, dims


def _build(dims):
    from concourse import bacc, bass, mybir, tile
    from concourse.masks import make_identity

    F = dims["F"]          # 128 input features
    H1 = dims["H1"]        # 4 heads, layer 1
    C = dims["C"]          # 64 channels per head
    D1 = H1 * C            # 256
    G1W = 2 * H1 + D1      # 264 = [ad(4) | as(4) | h(256)]
    G2W = 2 + C            # 66  = [ad2 | as2 | h2p]
    m1p, T1 = dims["m1p"], dims["T1"]
    rows_needed = dims["rows_needed"]
    NCH1 = m1p // P
    KCH = D1 // P          # 2 contraction chunks over 256
    slots_f = dict(dims["slots_f"])
    slots_i = dict(dims["slots_i"])
    slots_r = dict(dims["slots_r"])
    WF = max(b for _, b in slots_f.values())
    WI = max(b for _, b in slots_i.values())
    WR = max(b for _, b in slots_r.values())
    f32 = mybir.dt.float32
    bf16 = mybir.dt.bfloat16
    i32 = mybir.dt.int32
    AX0 = lambda ap: bass.IndirectOffsetOnAxis(ap=ap, axis=0)

    nc = bacc.Bacc("TRN2", target_bir_lowering=False, debug=False)

    xT_d = nc.declare_dram_parameter("xT", [F, m1p], f32, isOutput=False)
    pf_d = nc.declare_dram_parameter("packf", [P, WF], f32, isOutput=False)
    pi_d = nc.declare_dram_parameter("packi", [P, WI], i32, isOutput=False)
    pr_d = nc.declare_dram_parameter("packr", [P, WR], f32, isOutput=False)
    out_d = nc.declare_dram_parameter("out", [2, 1], f32, isOutput=True)

    g1_d = nc.dram_tensor("g1_tab", [m1p, G1W], f32)

    EQ = mybir.AluOpType.is_equal
    MAX = mybir.AluOpType.max
    MUL = mybir.AluOpType.mult
    Copy = mybir.ActivationFunctionType.Copy
    Exp = mybir.ActivationFunctionType.Exp
    Relu = mybir.ActivationFunctionType.Relu

    with tile.TileContext(nc) as tc:
        with (
            tc.tile_pool(name="const", bufs=1) as cp,
            tc.tile_pool(name="work", bufs=3) as wp,
            tc.tile_pool(name="fin", bufs=1) as fp,
            tc.tile_pool(name="psum", bufs=2, space="PSUM") as pp,
            tc.tile_pool(name="acc", bufs=1, space="PSUM") as ap_,
        ):
            # ---------------- constants / inputs into SBUF ----------------
            pk = cp.tile([P, WF], f32)
            nc.sync.dma_start(pk[:], pf_d[:])
            pki_s = cp.tile([P, WI], i32)
            nc.gpsimd.dma_start(pki_s[:], pi_d[:])
            pkr_s = cp.tile([P, WR], f32)
            nc.gpsimd.dma_start(pkr_s[:], pr_d[:])
            xT_s = cp.tile([F, m1p], f32)
            for ci in range(NCH1):
                nc.sync.dma_start(
                    xT_s[:, ci * P : (ci + 1) * P],
                    xT_d[:, ci * P : (ci + 1) * P],
                )

            def fsl(name, rows=P):
                a, b = slots_f[name]
                return pk[:rows, a:b]

            def isl(name):
                a, b = slots_i[name]
                return pki_s[:, a:b]

            def rsl(name):
                a, b = slots_r[name]
                return pkr_s[:, a:b]

            ident = cp.tile([P, P], f32)
            make_identity(nc, ident[:])
            iota_f = cp.tile([P, P], f32)
            nc.gpsimd.iota(
                iota_f[:], pattern=[[1, P]], base=0, channel_multiplier=0,
                allow_small_or_imprecise_dtypes=True,
            )
            iota_c = cp.tile([P, 1], f32)
            nc.gpsimd.iota(
                iota_c[:], pattern=[[0, 1]], base=0, channel_multiplier=1,
                allow_small_or_imprecise_dtypes=True,
            )

            # Ematᵀ for every layer-1 tile, from the replicated dstrel row
            emt_all = cp.tile([P, T1 * P], f32)
            nc.vector.tensor_scalar(
                emt_all[:], rsl("drep1"), iota_c[:, 0:1], None, EQ
            )
            # layer-2 one-hots from the replicated [src2 | dstrel2] rows
            l2r = rsl("l2rep")
            st2 = fp.tile([P, P], bf16)
            nc.vector.tensor_scalar(st2[:], l2r[:, :P], iota_c[:, 0:1], None, EQ)
            em2t = fp.tile([P, P], f32)
            nc.vector.tensor_scalar(em2t[:], l2r[:, P:], iota_c[:, 0:1], None, EQ)

            # ---------------- W1 @ [Ad | As]  (K = 256, 2 chunks) ----------
            pwa = pp.tile([F, 2 * H1], f32, tag="mm")
            for k in range(KCH):
                nc.tensor.matmul(
                    out=pwa[:], lhsT=fsl(f"w1T{k}"), rhs=fsl(f"ablk{k}"),
                    start=(k == 0), stop=(k == KCH - 1),
                )
            rhs1 = cp.tile([F, G1W], f32)
            nc.vector.tensor_copy(rhs1[:, : 2 * H1], pwa[:])
            nc.scalar.copy(rhs1[:, 2 * H1 :], fsl("w1"))

            # -------- G1 rows: [ad | as | h] = x @ [W1Ad | W1As | W1] ------
            # chunk-pipelined: each chunk's rows are written to DRAM as soon
            # as its matmul finishes, so gathers can start early
            gall = fp.tile([P, NCH1 * G1W], f32)
            for ci in range(NCH1):
                pg = pp.tile([P, G1W], f32, tag="pg")
                nc.tensor.matmul(
                    out=pg[:], lhsT=xT_s[:, ci * P : (ci + 1) * P],
                    rhs=rhs1[:], start=True, stop=True,
                )
                gsl = gall[:, ci * G1W : (ci + 1) * G1W]
                if ci % 2:
                    nc.scalar.copy(gsl, pg[:])
                else:
                    nc.vector.tensor_copy(gsl, pg[:])
                nc.sync.dma_start(g1_d[ci * P : (ci + 1) * P, :], gsl)
            # a_dst values for the destination slots: S2 is a prefix of S1,
            # so slot s lives in pre-phase chunk 0, row s, cols 0:H1.
            ad_slots = gall[:, 0:H1]

            # ---------------- layer-1 edge aggregation ---------------------
            agg1 = ap_.tile([P, D1 + H1], f32, tag="agg1")
            src1 = isl("src1")
            for t in range(T1):
                gs = wp.tile([P, G1W], f32, tag="gs")
                nc.gpsimd.indirect_dma_start(
                    out=gs[:], out_offset=None,
                    in_=g1_d[0 : rows_needed[t], :],
                    in_offset=AX0(src1[:, t : t + 1]),
                )
                adx_p = pp.tile([P, H1], f32, tag="mm")
                nc.tensor.matmul(
                    out=adx_p[:], lhsT=emt_all[:, t * P : (t + 1) * P],
                    rhs=ad_slots, start=True, stop=True,
                )
                e = wp.tile([P, H1], f32, tag="e")
                nc.vector.tensor_add(e[:], gs[:, H1 : 2 * H1], adx_p[:])
                es = wp.tile([P, H1], f32, tag="es")
                nc.vector.tensor_scalar_mul(es[:], e[:], 0.2)
                el = wp.tile([P, H1], f32, tag="el")
                nc.vector.tensor_tensor(out=el[:], in0=e[:], in1=es[:], op=MAX)
                pe = wp.tile([P, H1], f32, tag="pe")
                nc.scalar.activation(pe[:], el[:], Exp)
                rhs_t = wp.tile([P, D1 + H1], bf16, tag="rhs_t")
                nc.vector.tensor_tensor(
                    out=rhs_t[:, :D1].rearrange("p (h c) -> p h c", c=C),
                    in0=gs[:, 2 * H1 :].rearrange("p (h c) -> p h c", c=C),
                    in1=pe[:].rearrange("p (h o) -> p h o", o=1).to_broadcast(
                        [P, H1, C]
                    ),
                    op=MUL,
                )
                nc.scalar.activation(rhs_t[:, D1:], pe[:], Copy)
                em = wp.tile([P, P], bf16, tag="em")
                nc.vector.tensor_scalar(
                    em[:], iota_f[:], fsl("dstrel1")[:, t : t + 1], None, EQ
                )
                nc.tensor.matmul(
                    out=agg1[:], lhsT=em[:], rhs=rhs_t[:],
                    start=(t == 0), stop=(t == T1 - 1),
                )

            # ---------------- layer-1 finalize: h1r = relu(num/den + b1) ---
            den1 = fp.tile([P, H1], f32)
            nc.vector.tensor_scalar_add(den1[:], agg1[:, D1:], 1e-16)
            rec1 = fp.tile([P, H1], f32)
            nc.vector.reciprocal(rec1[:], den1[:])
            h1t = fp.tile([P, D1], f32)
            nc.vector.tensor_tensor(
                out=h1t[:].rearrange("p (h c) -> p h c", c=C),
                in0=agg1[:, :D1].rearrange("p (h c) -> p h c", c=C),
                in1=rec1[:].rearrange("p (h o) -> p h o", o=1).to_broadcast(
                    [P, H1, C]
                ),
                op=MUL,
            )
            h1b = fp.tile([P, D1], f32)
            nc.vector.tensor_add(h1b[:], h1t[:], fsl("b1bc"))
            h1r = fp.tile([P, D1], f32)
            nc.scalar.activation(h1r[:], h1b[:], Relu)

            # ---------------- layer-2 projection: G2 = [ad2|as2|h2p] -------
            pg2 = ap_.tile([P, G2W], f32, tag="pg2")
            for k in range(KCH):
                ptr = pp.tile([P, P], f32, tag="mm")
                nc.tensor.transpose(
                    out=ptr[:], in_=h1r[:, k * P : (k + 1) * P], identity=ident[:]
                )
                h1rT_k = wp.tile([P, P], f32, tag=f"h1rTk{k}")
                nc.vector.tensor_copy(h1rT_k[:], ptr[:])
                pwa2 = pp.tile([P, 2], f32, tag="mm")
                nc.tensor.matmul(
                    out=pwa2[:], lhsT=fsl("w2T", C)[:, k * P : (k + 1) * P],
                    rhs=fsl("a2", C), start=True, stop=True,
                )
                rhs2_k = wp.tile([P, G2W], f32, tag=f"rhs2k{k}")
                nc.vector.tensor_copy(rhs2_k[:, :2], pwa2[:])
                nc.scalar.copy(rhs2_k[:, 2:], fsl(f"w2_{k}"))
                nc.tensor.matmul(
                    out=pg2[:], lhsT=h1rT_k[:], rhs=rhs2_k[:],
                    start=(k == 0), stop=(k == KCH - 1),
                )
            g2sb = fp.tile([P, G2W], bf16)
            nc.scalar.copy(g2sb[:], pg2[:])
            g2ad = fp.tile([P, 1], f32)
            nc.vector.tensor_copy(g2ad[:], pg2[:, 0:1])

            # ------- layer-2 edge aggregation: fully on-chip (ball only) ---
            gs2_p = pp.tile([P, G2W], f32, tag="mm")
            nc.tensor.matmul(out=gs2_p[:], lhsT=st2[:], rhs=g2sb[:],
                             start=True, stop=False, skip_group_check=True)
            # accumulate the a_dst[ball] expansion straight onto the as2
            # column: gs2_p[:, 1] becomes e2 = as2[src] + ad2[dst]
            nc.tensor.matmul(out=gs2_p[:, 1:2], lhsT=em2t[:], rhs=g2ad[:],
                             start=False, stop=True, skip_group_check=True)
            es2 = fp.tile([P, 1], f32)
            nc.vector.tensor_scalar_mul(es2[:], gs2_p[:, 1:2], 0.2)
            el2 = fp.tile([P, 1], f32)
            nc.vector.tensor_tensor(
                out=el2[:], in0=gs2_p[:, 1:2], in1=es2[:], op=MAX
            )
            pe2 = fp.tile([P, 1], f32)
            nc.scalar.activation(pe2[:], el2[:], Exp)
            rhs2t = fp.tile([P, C + 1], bf16)
            nc.vector.tensor_tensor(
                out=rhs2t[:, :C], in0=gs2_p[:, 2:],
                in1=pe2[:].to_broadcast([P, C]), op=MUL,
            )
            nc.scalar.activation(rhs2t[:, C:], pe2[:], Copy)
            em2 = fp.tile([P, P], bf16)
            nc.vector.tensor_scalar(
                em2[:], iota_f[:], fsl("dstrel2")[:, 0:1], None, EQ
            )
            agg2 = ap_.tile([P, C + 1], f32, tag="agg2")
            nc.tensor.matmul(out=agg2[:], lhsT=em2[:], rhs=rhs2t[:],
                             start=True, stop=True)

            # ---------------- ball finalize + MLP --------------------------
            den2 = fp.tile([1, 1], f32)
            nc.vector.tensor_scalar_add(den2[:], agg2[0:1, C : C + 1], 1e-16)
            rec2 = fp.tile([1, 1], f32)
            nc.vector.reciprocal(rec2[:], den2[:])
            bf = fp.tile([1, C], f32)
            nc.scalar.activation(bf[:], agg2[0:1, :C], Copy, scale=rec2[:, 0:1])
            ptb = pp.tile([C, 1], f32, tag="mm")
            nc.tensor.transpose(out=ptb[:], in_=bf[:], identity=ident[0:1, 0:1])
            bfr = fp.tile([C, 1], f32)
            nc.scalar.activation(bfr[:], ptb[:], Relu, bias=fsl("b2col", C))

            pz = pp.tile([C // 2, 1], f32, tag="mm")
            nc.tensor.matmul(out=pz[:], lhsT=fsl("fc1w", C), rhs=bfr[:],
                             start=True, stop=True)
            zr = fp.tile([C // 2, 1], f32)
            nc.scalar.activation(zr[:], pz[:], Relu, bias=fsl("fc1b", C // 2))

            po = pp.tile([2, 1], f32, tag="mm")
            nc.tensor.matmul(out=po[:], lhsT=fsl("fc2w", C // 2), rhs=zr[:],
                             start=True, stop=True)
            osb = fp.tile([2, 1], f32)
            nc.vector.tensor_add(osb[:], po[:], fsl("fc2b", 2))
            nc.sync.dma_start(out_d[:], osb[:])

    nc.compile()
    return nc


def kernel(**inputs):
    from concourse.bass_utils import run_bass_kernel_spmd

    feed, dims = _host_preprocess(inputs)
    key = (dims["m1p"], dims["m2"], dims["T1"], dims["T2"], dims["rows_needed"])
    if key not in _CACHE:
        _CACHE[key] = _build(dims)
    nc = _CACHE[key]

    n_cores = 8
    in_maps = [dict(feed) for _ in range(n_cores)]
    res = run_bass_kernel_spmd(nc, in_maps, core_ids=list(range(n_cores)))
    out = np.asarray(res.results[0]["out"], dtype=np.float32).reshape(2)
    return out


# revision 12
# speedup vs baseline: 1.7624x; 1.0609x over previous
"""Trainium2 Bass kernel for nn_BallPredictorGNN.

The reference model is a 2-layer GAT over (N=20000, E=640000) followed by an
MLP applied to the LAST node only ("ball") — the output is a single [2] vector.
Only the ball's 2-hop dependency cone matters:

  layer 2 aggregates at the ball node only            (~25 in-edges)
  layer 1 aggregates at the ball's in-neighbours S2   (~25 nodes, ~800 edges)
  x @ W1 is needed for the sources of those edges S1  (~800 nodes)

Host side (pure data routing): extract the cone, sort edges by destination,
order source nodes by first use so the gather pipeline can start before the
whole projection table is written, build padded index tables, pack small
operands.  Device side (all FLOPs): dense projections via TensorE;
per-128-edge source-row gathers via indirect DMA from a DRAM table;
segment-softmax-aggregation via one-hot matmuls (bf16) with fused numerator /
denominator accumulation in PSUM
(alpha = exp(e)/sum exp(e) folded as  out = (sum exp(e)*h_src) / sum exp(e)).
The per-edge a_dst[dst] term is expanded on-chip as Ematᵀ.T @ ad_slots with
Ematᵀ built from a host-replicated index row.  Layer 2 runs entirely on-chip:
its "gather" is a one-hot matmul against the SBUF-resident G2 table.

The same program is replicated SPMD on all 8 NeuronCores (the cone is tiny, so
replication beats sharding + collectives); core 0's output is returned.
"""

import numpy as np

P = 128
_CACHE = {}


def _ceil(a, b):
    return -(-a // b)


def _pad_rows(a, n, fill=0):
    out = np.full((n,) + a.shape[1:], fill, a.dtype)
    out[: len(a)] = a
    return out


class _Packer:
    """Pack many small [p, w] operands into one [128, W] array, column-wise."""

    def __init__(self):
        self.cols = []
        self.pos = 0
        self.slots = {}

    def add(self, name, arr):
        p, w = arr.shape
        full = np.zeros((P, w), arr.dtype)
        full[:p] = arr
        self.cols.append(full)
        self.slots[name] = (self.pos, self.pos + w)
        self.pos += w

    def finish(self):
        return np.ascontiguousarray(np.concatenate(self.cols, axis=1))


def _host_preprocess(inputs):
    x = np.ascontiguousarray(np.asarray(inputs["x"], dtype=np.float32))
    ei = np.asarray(inputs["edge_index"]).astype(np.int64)
    N = x.shape[0]
    F = x.shape[1]
    ball = N - 1
    src, dst = ei[0], ei[1]

    # ---- layer-2 cone: edges into the ball (+ self loop) --------------------
    sel2 = dst == ball
    e2s = np.concatenate([src[sel2], [ball]])
    uniq = np.unique(e2s)
    S2 = np.concatenate([[ball], uniq[uniq != ball]]).astype(np.int64)
    m2 = len(S2)
    assert m2 <= 127, f"ball in-neighbourhood too large for one dst block: {m2}"

    # ---- layer-1 cone: edges into S2 (+ self loops for S2) ------------------
    in_S2 = np.zeros(N, dtype=bool)
    in_S2[S2] = True
    sel1 = in_S2[dst]
    e1s = np.concatenate([src[sel1], S2])
    e1d = np.concatenate([dst[sel1], S2])

    # sort edges by destination slot (S2 order)
    loc2 = np.full(N, -1, dtype=np.int64)
    loc2[S2] = np.arange(m2)
    d_slot = loc2[e1d]
    order = np.argsort(d_slot, kind="stable")
    e1s, d_slot = e1s[order], d_slot[order]
    n1 = len(e1s)
    T1 = _ceil(n1, P)
    n1p = T1 * P

    # order S1 = [S2 | rest-by-first-use-in-edge-stream] so that gather tile t
    # only touches low G1 rows -> gathers overlap the projection pipeline
    seen = in_S2.copy()
    rest = []
    for s in e1s:
        if not seen[s]:
            seen[s] = True
            rest.append(s)
    S1 = np.concatenate([S2, np.asarray(rest, dtype=np.int64)])
    m1 = len(S1)
    m1p = _ceil(m1, P) * P
    loc1 = np.full(N, -1, dtype=np.int64)
    loc1[S1] = np.arange(m1)
    s_loc = loc1[e1s]

    # rows of G1 needed by each gather tile (for sliced dependency ranges)
    smax = np.maximum.accumulate(
        np.max(_pad_rows(s_loc, n1p).reshape(T1, P), axis=1)
    )
    rows_needed = np.minimum(
        (_ceil(int(m2), P) + 0) * 0 + (smax // P + 1) * P, m1p
    ).astype(int)

    # per-tile column layout [P, T]: element (p, t) = edge t*P + p
    def cols(a, n_pad, fill, dt):
        return np.ascontiguousarray(
            _pad_rows(a.astype(dt), n_pad, fill).reshape(-1, P).T
        )

    def row(a, n_pad, fill, dt):
        return _pad_rows(a.astype(dt), n_pad, fill)[None, :]

    s2_loc = loc1[e2s]  # all < m2
    n2 = len(s2_loc)
    T2 = _ceil(n2, P)
    assert T2 == 1, f"layer-2 edge count exceeds one tile: {n2}"
    n2p = T2 * P

    pki = _Packer()
    pki.add("src1", cols(s_loc, n1p, 0, np.int32))

    # ---- dense operands -----------------------------------------------------
    xT = np.ascontiguousarray(_pad_rows(x[S1], m1p).T)  # [F, m1p]

    W1 = np.asarray(inputs["W1"], np.float32)  # [F, 4*64]
    a_src1 = np.asarray(inputs["a_src1"], np.float32)  # [4, 64]
    a_dst1 = np.asarray(inputs["a_dst1"], np.float32)
    H1, C = a_src1.shape
    D1 = H1 * C
    ablk = np.zeros((D1, 2 * H1), np.float32)  # [256, 8] = [Ad | As]
    for h in range(H1):
        ablk[h * C : (h + 1) * C, h] = a_dst1[h]
        ablk[h * C : (h + 1) * C, H1 + h] = a_src1[h]

    W2 = np.asarray(inputs["W2"], np.float32)  # [256, 64]
    a2 = np.stack(
        [np.asarray(inputs["a_dst2"], np.float32)[0],
         np.asarray(inputs["a_src2"], np.float32)[0]],
        axis=1,
    )  # [64, 2] = [a_dst | a_src]

    pkf = _Packer()
    pkf.add("w1", W1)
    W1T = np.ascontiguousarray(W1.T)
    for k in range(D1 // P):
        pkf.add(f"w1T{k}", W1T[k * P : (k + 1) * P])
        pkf.add(f"ablk{k}", ablk[k * P : (k + 1) * P])
        pkf.add(f"w2_{k}", W2[k * P : (k + 1) * P])
    pkf.add("b1bc", np.broadcast_to(np.asarray(inputs["b1"], np.float32), (P, D1)))
    pkf.add("w2T", np.ascontiguousarray(W2.T))
    pkf.add("a2", a2)
    pkf.add("b2col", np.asarray(inputs["b2"], np.float32)[:, None])
    pkf.add("fc1w", np.ascontiguousarray(np.asarray(inputs["fc1_w"], np.float32)))
    pkf.add("fc1b", np.asarray(inputs["fc1_b"], np.float32)[:, None])
    pkf.add("fc2w", np.ascontiguousarray(np.asarray(inputs["fc2_w"], np.float32)))
    pkf.add("fc2b", np.asarray(inputs["fc2_b"], np.float32)[:, None])
    pkf.add("dstrel1", cols(d_slot, n1p, m2, np.float32))  # pad -> garbage slot
    pkf.add("dstrel2", cols(np.zeros(n2, np.int64), n2p, 1, np.float32))

    pkr = _Packer()
    pkr.add("drep1", np.broadcast_to(
        row(d_slot, n1p, m2, np.float32), (P, n1p)))
    pkr.add("l2rep", np.broadcast_to(np.concatenate(
        [row(s2_loc, n2p, 0, np.float32),
         row(np.zeros(n2), n2p, 1, np.float32)], axis=1), (P, 2 * n2p)))

    feed = {
        "xT": xT, "packf": pkf.finish(), "packi": pki.finish(),
        "packr": pkr.finish(),
    }
    dims = dict(
        F=F, H1=H1, C=C, m1p=m1p, m2=m2, T1=T1, T2=T2,
        rows_needed=tuple(int(r) for r in rows_needed),
        slots_f=tuple(sorted(pkf.slots.items())),
        slots_i=tuple(sorted(pki.slots.items())),
        slots_r=tuple(sorted(pkr.slots.items())),
    )
    return feed, dims


def _build(dims):
    from concourse import bacc, bass, mybir, tile
    from concourse.masks import make_identity

    F = dims["F"]          # 128 input features
    H1 = dims["H1"]        # 4 heads, layer 1
    C = dims["C"]          # 64 channels per head
    D1 = H1 * C            # 256
    G1W = 2 * H1 + D1      # 264 = [ad(4) | as(4) | h(256)]
    G2W = 2 + C            # 66  = [ad2 | as2 | h2p]
    m1p, T1 = dims["m1p"], dims["T1"]
    rows_needed = dims["rows_needed"]
    NCH1 = m1p // P
    KCH = D1 // P          # 2 contraction chunks over 256
    slots_f = dict(dims["slots_f"])
    slots_i = dict(dims["slots_i"])
    slots_r = dict(dims["slots_r"])
    WF = max(b for _, b in slots_f.values())
    WI = max(b for _, b in slots_i.values())
    WR = max(b for _, b in slots_r.values())
    f32 = mybir.dt.float32
    bf16 = mybir.dt.bfloat16
    i32 = mybir.dt.int32
    AX0 = lambda ap: bass.IndirectOffsetOnAxis(ap=ap, axis=0)

    nc = bacc.Bacc("TRN2", target_bir_lowering=False, debug=False)

    xT_d = nc.declare_dram_parameter("xT", [F, m1p], f32, isOutput=False)
    pf_d = nc.declare_dram_parameter("packf", [P, WF], f32, isOutput=False)
    pi_d = nc.declare_dram_parameter("packi", [P, WI], i32, isOutput=False)
    pr_d = nc.declare_dram_parameter("packr", [P, WR], f32, isOutput=False)
    out_d = nc.declare_dram_parameter("out", [2, 1], f32, isOutput=True)

    g1_d = nc.dram_tensor("g1_tab", [m1p, G1W], f32)

    EQ = mybir.AluOpType.is_equal
    MAX = mybir.AluOpType.max
    MUL = mybir.AluOpType.mult
    Copy = mybir.ActivationFunctionType.Copy
    Exp = mybir.ActivationFunctionType.Exp
    Relu = mybir.ActivationFunctionType.Relu

    with tile.TileContext(nc) as tc:
        with (
            tc.tile_pool(name="const", bufs=1) as cp,
            tc.tile_pool(name="work", bufs=3) as wp,
            tc.tile_pool(name="fin", bufs=1) as fp,
            tc.tile_pool(name="psum", bufs=2, space="PSUM") as pp,
            tc.tile_pool(name="acc", bufs=1, space="PSUM") as ap_,
        ):
            # ---------------- constants / inputs into SBUF ----------------
            pk = cp.tile([P, WF], f32)
            nc.sync.dma_start(pk[:], pf_d[:])
            pki_s = cp.tile([P, WI], i32)
            nc.gpsimd.dma_start(pki_s[:], pi_d[:])
            pkr_s = cp.tile([P, WR], f32)
            nc.gpsimd.dma_start(pkr_s[:], pr_d[:])
            xT_s = cp.tile([F, m1p], f32)
            for ci in range(NCH1):
                eng = nc.sync if ci % 2 == 0 else nc.gpsimd
                eng.dma_start(
                    xT_s[:, ci * P : (ci + 1) * P],
                    xT_d[:, ci * P : (ci + 1) * P],
                )

            def fsl(name, rows=P):
                a, b = slots_f[name]
                return pk[:rows, a:b]

            def isl(name):
                a, b = slots_i[name]
                return pki_s[:, a:b]

            def rsl(name):
                a, b = slots_r[name]
                return pkr_s[:, a:b]

            ident = cp.tile([P, P], f32)
            make_identity(nc, ident[:])
            iota_f = cp.tile([P, P], f32)
            nc.gpsimd.iota(
                iota_f[:], pattern=[[1, P]], base=0, channel_multiplier=0,
                allow_small_or_imprecise_dtypes=True,
            )
            iota_c = cp.tile([P, 1], f32)
            nc.gpsimd.iota(
                iota_c[:], pattern=[[0, 1]], base=0, channel_multiplier=1,
                allow_small_or_imprecise_dtypes=True,
            )

            # ---------------- W1 @ [Ad | As]  (K = 256, 2 chunks) ----------
            pwa = pp.tile([F, 2 * H1], f32, tag="mm")
            for k in range(KCH):
                nc.tensor.matmul(
                    out=pwa[:], lhsT=fsl(f"w1T{k}"), rhs=fsl(f"ablk{k}"),
                    start=(k == 0), stop=(k == KCH - 1),
                )
            rhs1 = cp.tile([F, G1W], f32)
            nc.vector.tensor_copy(rhs1[:, : 2 * H1], pwa[:])
            nc.scalar.copy(rhs1[:, 2 * H1 :], fsl("w1"))

            # -------- G1 rows: [ad | as | h] = x @ [W1Ad | W1As | W1] ------
            # chunk-pipelined: each chunk's rows are written to DRAM as soon
            # as its matmul finishes, so gathers can start early
            gall = fp.tile([P, NCH1 * G1W], f32)
            for ci in range(NCH1):
                pg = pp.tile([P, G1W], f32, tag="pg")
                nc.tensor.matmul(
                    out=pg[:], lhsT=xT_s[:, ci * P : (ci + 1) * P],
                    rhs=rhs1[:], start=True, stop=True,
                )
                gsl = gall[:, ci * G1W : (ci + 1) * G1W]
                if ci % 2:
                    nc.scalar.copy(gsl, pg[:])
                else:
                    nc.vector.tensor_copy(gsl, pg[:])
                nc.sync.dma_start(g1_d[ci * P : (ci + 1) * P, :], gsl)
            # a_dst values for the destination slots: S2 is a prefix of S1,
            # so slot s lives in pre-phase chunk 0, row s, cols 0:H1.
            ad_slots = gall[:, 0:H1]

            # Ematᵀ for every layer-1 tile, from the replicated dstrel row
            emt_all = cp.tile([P, T1 * P], f32)
            nc.vector.tensor_scalar(
                emt_all[:], rsl("drep1"), iota_c[:, 0:1], None, EQ
            )
            # layer-2 one-hots from the replicated [src2 | dstrel2] rows
            l2r = rsl("l2rep")
            st2 = fp.tile([P, P], bf16)
            nc.vector.tensor_scalar(st2[:], l2r[:, :P], iota_c[:, 0:1], None, EQ)
            em2t = fp.tile([P, P], f32)
            nc.vector.tensor_scalar(em2t[:], l2r[:, P:], iota_c[:, 0:1], None, EQ)
            # layer-2 projection RHS [W2A2 | W2] per K-chunk (no h1r dependency)
            rhs2 = []
            for k in range(KCH):
                pwa2 = pp.tile([P, 2], f32, tag="mm")
                nc.tensor.matmul(
                    out=pwa2[:], lhsT=fsl("w2T", C)[:, k * P : (k + 1) * P],
                    rhs=fsl("a2", C), start=True, stop=True,
                )
                rhs2_k = cp.tile([P, G2W], f32, name=f"rhs2_{k}")
                nc.vector.tensor_copy(rhs2_k[:, :2], pwa2[:])
                nc.scalar.copy(rhs2_k[:, 2:], fsl(f"w2_{k}"))
                rhs2.append(rhs2_k)

            # ------- layer-1 edge aggregation (two 128-edge tiles/group) ----
            agg1 = ap_.tile([P, D1 + H1], f32, tag="agg1")
            src1 = isl("src1")
            W1R = D1 + H1  # 260: per-tile rhs width [msg(256) | p(4)]
            t = 0
            while t < T1:
                G = min(2, T1 - t)  # tiles in this group
                gs = wp.tile([P, 2 * G1W], f32, tag="gs")
                for j in range(G):
                    nc.gpsimd.indirect_dma_start(
                        out=gs[:, j * G1W : (j + 1) * G1W], out_offset=None,
                        in_=g1_d[0 : rows_needed[t + j], :],
                        in_offset=AX0(src1[:, t + j : t + j + 1]),
                    )
                adx_p = pp.tile([P, 2 * H1], f32, tag="mm")
                for j in range(G):
                    nc.tensor.matmul(
                        out=adx_p[:, j * H1 : (j + 1) * H1],
                        lhsT=emt_all[:, (t + j) * P : (t + j + 1) * P],
                        rhs=ad_slots, start=True, stop=True,
                        skip_group_check=True,
                    )
                gsv = gs[:].rearrange("p (t w) -> p t w", w=G1W)
                e = wp.tile([P, G * H1], f32, tag="e")
                nc.vector.tensor_add(
                    e[:].rearrange("p (t h) -> p t h", h=H1),
                    gsv[:, :G, H1 : 2 * H1],
                    adx_p[:, : G * H1].rearrange("p (t h) -> p t h", h=H1),
                )
                es = wp.tile([P, G * H1], f32, tag="es")
                nc.vector.tensor_scalar_mul(es[:], e[:], 0.2)
                el = wp.tile([P, G * H1], f32, tag="el")
                nc.vector.tensor_tensor(out=el[:], in0=e[:], in1=es[:], op=MAX)
                pe = wp.tile([P, G * H1], f32, tag="pe")
                nc.scalar.activation(pe[:], el[:], Exp)
                rhs_t = wp.tile([P, 2 * W1R], bf16, tag="rhs_t")
                rhv = rhs_t[:].rearrange("p (t w) -> p t w", w=W1R)
                nc.vector.tensor_tensor(
                    out=rhv[:, :G, :D1].rearrange("p t (h c) -> p t h c", c=C),
                    in0=gsv[:, :G, 2 * H1 :].rearrange(
                        "p t (h c) -> p t h c", c=C
                    ),
                    in1=pe[:].rearrange(
                        "p (t h o) -> p t h o", h=H1, o=1
                    ).to_broadcast([P, G, H1, C]),
                    op=MUL,
                )
                nc.scalar.activation(
                    rhv[:, :G, D1:],
                    pe[:].rearrange("p (t h) -> p t h", h=H1),
                    Copy,
                )
                for j in range(G):
                    em = wp.tile([P, P], bf16, tag="em")
                    nc.vector.tensor_scalar(
                        em[:], iota_f[:],
                        fsl("dstrel1")[:, t + j : t + j + 1], None, EQ,
                    )
                    nc.tensor.matmul(
                        out=agg1[:], lhsT=em[:],
                        rhs=rhv[:, t + j - t, :],
                        start=(t + j == 0), stop=(t + j == T1 - 1),
                    )
                t += G

            # ---------------- layer-1 finalize: h1r = relu(num/den + b1) ---
            den1 = fp.tile([P, H1], f32)
            nc.vector.tensor_scalar_add(den1[:], agg1[:, D1:], 1e-16)
            rec1 = fp.tile([P, H1], f32)
            nc.vector.reciprocal(rec1[:], den1[:])
            h1t = fp.tile([P, D1], f32)
            nc.vector.tensor_tensor(
                out=h1t[:].rearrange("p (h c) -> p h c", c=C),
                in0=agg1[:, :D1].rearrange("p (h c) -> p h c", c=C),
                in1=rec1[:].rearrange("p (h o) -> p h o", o=1).to_broadcast(
                    [P, H1, C]
                ),
                op=MUL,
            )
            h1b = fp.tile([P, D1], f32)
            nc.vector.tensor_add(h1b[:], h1t[:], fsl("b1bc"))
            h1r = fp.tile([P, D1], f32)
            nc.scalar.activation(h1r[:], h1b[:], Relu)

            # ---------------- layer-2 projection: G2 = [ad2|as2|h2p] -------
            pg2 = ap_.tile([P, G2W], f32, tag="pg2")
            for k in range(KCH):
                ptr = pp.tile([P, P], f32, tag="mm")
                nc.tensor.transpose(
                    out=ptr[:], in_=h1r[:, k * P : (k + 1) * P], identity=ident[:]
                )
                h1rT_k = wp.tile([P, P], f32, tag=f"h1rTk{k}")
                nc.vector.tensor_copy(h1rT_k[:], ptr[:])
                nc.tensor.matmul(
                    out=pg2[:], lhsT=h1rT_k[:], rhs=rhs2[k][:],
                    start=(k == 0), stop=(k == KCH - 1),
                )
            g2sb = fp.tile([P, G2W], bf16)
            nc.scalar.copy(g2sb[:], pg2[:])
            g2ad = fp.tile([P, 1], f32)
            nc.vector.tensor_copy(g2ad[:], pg2[:, 0:1])

            # ------- layer-2 edge aggregation: fully on-chip (ball only) ---
            gs2_p = pp.tile([P, G2W], f32, tag="mm")
            nc.tensor.matmul(out=gs2_p[:], lhsT=st2[:], rhs=g2sb[:],
                             start=True, stop=False, skip_group_check=True)
            # accumulate the a_dst[ball] expansion straight onto the as2
            # column: gs2_p[:, 1] becomes e2 = as2[src] + ad2[dst]
            nc.tensor.matmul(out=gs2_p[:, 1:2], lhsT=em2t[:], rhs=g2ad[:],
                             start=False, stop=True, skip_group_check=True)
            es2 = fp.tile([P, 1], f32)
            nc.vector.tensor_scalar_mul(es2[:], gs2_p[:, 1:2], 0.2)
            el2 = fp.tile([P, 1], f32)
            nc.vector.tensor_tensor(
                out=el2[:], in0=gs2_p[:, 1:2], in1=es2[:], op=MAX
            )
            pe2 = fp.tile([P, 1], f32)
            nc.scalar.activation(pe2[:], el2[:], Exp)
            rhs2t = fp.tile([P, C + 1], bf16)
            nc.vector.tensor_tensor(
                out=rhs2t[:, :C], in0=gs2_p[:, 2:],
                in1=pe2[:].to_broadcast([P, C]), op=MUL,
            )
            nc.scalar.activation(rhs2t[:, C:], pe2[:], Copy)
            em2 = fp.tile([P, P], bf16)
            nc.vector.tensor_scalar(
                em2[:], iota_f[:], fsl("dstrel2")[:, 0:1], None, EQ
            )
            agg2 = ap_.tile([P, C + 1], f32, tag="agg2")
            nc.tensor.matmul(out=agg2[:], lhsT=em2[:], rhs=rhs2t[:],
                             start=True, stop=True)

            # ---------------- ball finalize + MLP --------------------------
            den2 = fp.tile([1, 1], f32)
            nc.vector.tensor_scalar_add(den2[:], agg2[0:1, C : C + 1], 1e-16)
            rec2 = fp.tile([1, 1], f32)
            nc.vector.reciprocal(rec2[:], den2[:])
            bf = fp.tile([1, C], f32)
            nc.scalar.activation(bf[:], agg2[0:1, :C], Copy, scale=rec2[:, 0:1])
            ptb = pp.tile([C, 1], f32, tag="mm")
            nc.tensor.transpose(out=ptb[:], in_=bf[:], identity=ident[0:1, 0:1])
            bfr = fp.tile([C, 1], f32)
            nc.scalar.activation(bfr[:], ptb[:], Relu, bias=fsl("b2col", C))

            pz = pp.tile([C // 2, 1], f32, tag="mm")
            nc.tensor.matmul(out=pz[:], lhsT=fsl("fc1w", C), rhs=bfr[:],
                             start=True, stop=True)
            zr = fp.tile([C // 2, 1], f32)
            nc.scalar.activation(zr[:], pz[:], Relu, bias=fsl("fc1b", C // 2))

            po = pp.tile([2, 1], f32, tag="mm")
            nc.tensor.matmul(out=po[:], lhsT=fsl("fc2w", C // 2), rhs=zr[:],
                             start=True, stop=True)
            osb = fp.tile([2, 1], f32)
            nc.vector.tensor_add(osb[:], po[:], fsl("fc2b", 2))
            nc.sync.dma_start(out_d[:], osb[:])

    nc.compile()
    return nc


def kernel(**inputs):
    from concourse.bass_utils import run_bass_kernel_spmd

    feed, dims = _host_preprocess(inputs)
    key = (dims["m1p"], dims["m2"], dims["T1"], dims["T2"], dims["rows_needed"])
    if key not in _CACHE:
        _CACHE[key] = _build(dims)
    nc = _CACHE[key]

    n_cores = 8
    in_maps = [dict(feed) for _ in range(n_cores)]
    res = run_bass_kernel_spmd(nc, in_maps, core_ids=list(range(n_cores)))
    out = np.asarray(res.results[0]["out"], dtype=np.float32).reshape(2)
    return out


# revision 14
# speedup vs baseline: 2.1558x; 1.2232x over previous
"""Trainium2 Bass kernel for nn_BallPredictorGNN.

The reference model is a 2-layer GAT over (N=20000, E=640000) followed by an
MLP applied to the LAST node only ("ball") — the output is a single [2] vector.
Only the ball's 2-hop dependency cone matters:

  layer 2 aggregates at the ball node only            (~25 in-edges)
  layer 1 aggregates at the ball's in-neighbours S2   (~25 nodes, ~800 edges)
  x @ W1 is needed for the sources of those edges     (~800 edges)

Host side (pure data routing): extract the cone and lay layer-1 edges out on a
[128 partitions x K chunks] grid, where each partition serves one destination
node (high-degree destinations get several partitions, merged at the end by a
single one-hot matmul).  The source features are replicated per edge-slot into
the xT operand, so the projection matmul directly produces per-edge rows
[ad | as | h] = x_src @ [W1Ad | W1As | W1] in the right partition — no
gather, no DRAM round-trip, no indirect DMA anywhere.

Device side (all FLOPs): per chunk, one TensorE matmul projects 128 edges;
VectorE computes e = as + ad + mask, leaky-relu, and accumulates
msg += h * exp(e) and den += exp(e) along the free axis
(alpha = exp(e)/sum exp(e) folded as  out = (sum exp(e)*h_src) / sum exp(e);
masked/padded slots contribute exp(-1e30) = 0).  Layer 2 (ball only) runs
fully on-chip with one-hot matmuls against the SBUF-resident projection.

The same program is replicated SPMD on all 8 NeuronCores (the cone is tiny, so
replication beats sharding + collectives); core 0's output is returned.
"""

import numpy as np

P = 128
_CACHE = {}


def _ceil(a, b):
    return -(-a // b)


def _pad_rows(a, n, fill=0):
    out = np.full((n,) + a.shape[1:], fill, a.dtype)
    out[: len(a)] = a
    return out


class _Packer:
    """Pack many small [p, w] operands into one [128, W] array, column-wise."""

    def __init__(self, dtype=np.float32):
        self.cols = []
        self.pos = 0
        self.slots = {}
        self.dtype = dtype

    def add(self, name, arr):
        p, w = arr.shape
        full = np.zeros((P, w), self.dtype)
        full[:p] = arr
        self.cols.append(full)
        self.slots[name] = (self.pos, self.pos + w)
        self.pos += w

    def finish(self):
        return np.ascontiguousarray(np.concatenate(self.cols, axis=1))


NEG = np.float32(-1e30)


def _host_preprocess(inputs):
    x = np.asarray(inputs["x"], dtype=np.float32)
    ei = np.asarray(inputs["edge_index"]).astype(np.int64)
    N = x.shape[0]
    F = x.shape[1]
    ball = N - 1
    src, dst = ei[0], ei[1]

    # ---- layer-2 cone: edges into the ball (+ self loop) --------------------
    sel2 = dst == ball
    e2s = np.concatenate([src[sel2], [ball]])
    uniq = np.unique(e2s)
    S2 = np.concatenate([[ball], uniq[uniq != ball]]).astype(np.int64)
    m2 = len(S2)
    assert m2 <= 127, f"ball in-neighbourhood too large for one dst block: {m2}"

    # ---- layer-1 edge grid: [partition, chunk] ------------------------------
    in_S2 = np.zeros(N, dtype=bool)
    in_S2[S2] = True
    sel1 = in_S2[dst]
    l1s, l1d = src[sel1], dst[sel1]  # self loops handled separately

    # per-destination source lists (excluding the self loop)
    loc2 = np.full(N, -1, dtype=np.int64)
    loc2[S2] = np.arange(m2)
    by_dst = [[] for _ in range(m2)]
    for s, d in zip(l1s, loc2[l1d]):
        by_dst[d].append(s)

    # choose K (chunks) so all partition groups fit in 128 partitions
    K = 2
    while sum(max(1, _ceil(len(g), K - 1)) for g in by_dst) > P:
        K += 1
    K = max(K, 3)

    grid_src = np.zeros((P, K), dtype=np.int64)  # source node per slot
    grid_valid = np.zeros((P, K), dtype=bool)
    slotmap = np.full(P, P - 1, dtype=np.int64)  # partition -> dst slot
    p = 0
    for sidx in range(m2):
        g = by_dst[sidx]
        v = S2[sidx]
        nparts = max(1, _ceil(len(g), K - 1))
        for gi in range(nparts):
            grid_src[p, 0] = v  # self loop (duplicates masked)
            grid_valid[p, 0] = gi == 0
            chunk_edges = g[gi * (K - 1) : (gi + 1) * (K - 1)]
            for j, s in enumerate(chunk_edges):
                grid_src[p, 1 + j] = s
                grid_valid[p, 1 + j] = True
            slotmap[p] = sidx
            p += 1
    assert p <= P

    # xT: [F, K*128] with column k*128+q = x[grid_src[q, k]] (bf16).
    # Chunk-0 columns keep their features even when masked: secondary
    # partitions read a_dst[dst] from their (duplicate) self-loop row.
    zero_slots = ~grid_valid
    zero_slots[:, 0] = False
    xg = x[grid_src.T.reshape(-1)]  # [K*128, F]
    xg[zero_slots.T.reshape(-1)] = 0
    import ml_dtypes

    xT = np.ascontiguousarray(xg.T.astype(ml_dtypes.bfloat16))  # [F, K*128]

    admask = np.where(grid_valid, np.float32(0), NEG).astype(np.float32)  # [P,K]
    pmapcol = slotmap[:, None].astype(np.float32)  # [P,1]

    # ---- layer-2 index rows -------------------------------------------------
    s2_loc = loc2[e2s]  # all < m2
    n2 = len(s2_loc)
    T2 = _ceil(n2, P)
    assert T2 == 1, f"layer-2 edge count exceeds one tile: {n2}"
    n2p = T2 * P

    def row(a, n_pad, fill, dt):
        return _pad_rows(a.astype(dt), n_pad, fill)[None, :]

    # ---- dense operands -----------------------------------------------------
    W1 = np.asarray(inputs["W1"], np.float32)  # [F, 4*64]
    a_src1 = np.asarray(inputs["a_src1"], np.float32)  # [4, 64]
    a_dst1 = np.asarray(inputs["a_dst1"], np.float32)
    H1, C = a_src1.shape
    D1 = H1 * C
    ablk = np.zeros((D1, 2 * H1), np.float32)  # [256, 8] = [Ad | As]
    for h in range(H1):
        ablk[h * C : (h + 1) * C, h] = a_dst1[h]
        ablk[h * C : (h + 1) * C, H1 + h] = a_src1[h]

    W2 = np.asarray(inputs["W2"], np.float32)  # [256, 64]
    a2 = np.stack(
        [np.asarray(inputs["a_dst2"], np.float32)[0],
         np.asarray(inputs["a_src2"], np.float32)[0]],
        axis=1,
    )  # [64, 2] = [a_dst | a_src]

    pkf = _Packer()
    pkf.add("w1", W1)
    W1T = np.ascontiguousarray(W1.T)
    for k in range(D1 // P):
        pkf.add(f"w1T{k}", W1T[k * P : (k + 1) * P])
        pkf.add(f"ablk{k}", ablk[k * P : (k + 1) * P])
        pkf.add(f"w2_{k}", W2[k * P : (k + 1) * P])
    pkf.add("b1bc", np.broadcast_to(np.asarray(inputs["b1"], np.float32), (P, D1)))
    pkf.add("w2T", np.ascontiguousarray(W2.T))
    pkf.add("a2", a2)
    pkf.add("b2col", np.asarray(inputs["b2"], np.float32)[:, None])
    pkf.add("fc1w", np.ascontiguousarray(np.asarray(inputs["fc1_w"], np.float32)))
    pkf.add("fc1b", np.asarray(inputs["fc1_b"], np.float32)[:, None])
    pkf.add("fc2w", np.ascontiguousarray(np.asarray(inputs["fc2_w"], np.float32)))
    pkf.add("fc2b", np.asarray(inputs["fc2_b"], np.float32)[:, None])
    pkf.add("admask", admask)
    pkf.add("pmapcol", pmapcol)
    pkf.add("dstrel2", np.ascontiguousarray(
        _pad_rows(np.zeros(n2, np.float32), n2p, 1)[:, None]))
    pkf.add("l2rep", np.broadcast_to(np.concatenate(
        [row(s2_loc, n2p, 0, np.float32),
         row(np.zeros(n2), n2p, 1, np.float32)], axis=1), (P, 2 * n2p)))

    feed = {"xT": xT, "packf": pkf.finish()}
    dims = dict(
        F=F, H1=H1, C=C, K=K, m2=m2, T2=T2,
        slots_f=tuple(sorted(pkf.slots.items())),
    )
    return feed, dims


def _build(dims):
    from concourse import bacc, bass, mybir, tile
    from concourse.masks import make_identity

    F = dims["F"]          # 128 input features
    H1 = dims["H1"]        # 4 heads, layer 1
    C = dims["C"]          # 64 channels per head
    D1 = H1 * C            # 256
    G1W = 2 * H1 + D1      # 264 = [ad(4) | as(4) | h(256)]
    G2W = 2 + C            # 66  = [ad2 | as2 | h2p]
    K = dims["K"]          # layer-1 chunks (edge slots per partition)
    KCH = D1 // P          # 2 contraction chunks over 256
    slots_f = dict(dims["slots_f"])
    WF = max(b for _, b in slots_f.values())
    f32 = mybir.dt.float32
    bf16 = mybir.dt.bfloat16

    nc = bacc.Bacc("TRN2", target_bir_lowering=False, debug=False)

    xT_d = nc.declare_dram_parameter("xT", [F, K * P], bf16, isOutput=False)
    pf_d = nc.declare_dram_parameter("packf", [P, WF], f32, isOutput=False)
    out_d = nc.declare_dram_parameter("out", [2, 1], f32, isOutput=True)

    EQ = mybir.AluOpType.is_equal
    MAX = mybir.AluOpType.max
    ADD = mybir.AluOpType.add
    MUL = mybir.AluOpType.mult
    Copy = mybir.ActivationFunctionType.Copy
    Exp = mybir.ActivationFunctionType.Exp
    Relu = mybir.ActivationFunctionType.Relu

    with tile.TileContext(nc) as tc:
        with (
            tc.tile_pool(name="const", bufs=1) as cp,
            tc.tile_pool(name="work", bufs=3) as wp,
            tc.tile_pool(name="fin", bufs=1) as fp,
            tc.tile_pool(name="psum", bufs=2, space="PSUM") as pp,
            tc.tile_pool(name="pgp", bufs=3, space="PSUM") as pgp,
            tc.tile_pool(name="acc", bufs=1, space="PSUM") as ap_,
        ):
            # ---------------- inputs into SBUF -----------------------------
            pk = cp.tile([P, WF], f32)
            nc.sync.dma_start(pk[:], pf_d[:])
            xT_s = cp.tile([F, K * P], bf16)
            for ci in range(K):
                eng = nc.sync if ci % 2 == 0 else nc.gpsimd
                eng.dma_start(
                    xT_s[:, ci * P : (ci + 1) * P],
                    xT_d[:, ci * P : (ci + 1) * P],
                )

            def fsl(name, rows=P):
                a, b = slots_f[name]
                return pk[:rows, a:b]

            ident = cp.tile([P, P], f32)
            make_identity(nc, ident[:])
            iota_f = cp.tile([P, P], f32)
            nc.gpsimd.iota(
                iota_f[:], pattern=[[1, P]], base=0, channel_multiplier=0,
                allow_small_or_imprecise_dtypes=True,
            )
            iota_c = cp.tile([P, 1], f32)
            nc.gpsimd.iota(
                iota_c[:], pattern=[[0, 1]], base=0, channel_multiplier=1,
                allow_small_or_imprecise_dtypes=True,
            )

            # ---------------- W1 @ [Ad | As]  (K = 256, 2 chunks) ----------
            pwa = pp.tile([F, 2 * H1], f32, tag="mm")
            for k in range(KCH):
                nc.tensor.matmul(
                    out=pwa[:], lhsT=fsl(f"w1T{k}"), rhs=fsl(f"ablk{k}"),
                    start=(k == 0), stop=(k == KCH - 1),
                )
            rhs1 = cp.tile([F, G1W], bf16)
            nc.vector.tensor_copy(rhs1[:, : 2 * H1], pwa[:])
            nc.scalar.copy(rhs1[:, 2 * H1 :], fsl("w1"))

            # ---------------- layer-1 edge chunks --------------------------
            # chunk k: project 128 edge slots, then accumulate
            #   acc[:, :256] += h * exp(e)   acc[:, 256:260] += exp(e)
            acc = fp.tile([P, D1 + H1], f32)
            nc.gpsimd.memset(acc[:], 0.0)
            ad_part = fp.tile([P, H1], f32)
            admix = fp.tile([P, K * H1], f32)
            for k in range(K):
                pg = pgp.tile([P, G1W], f32, tag="pg")
                nc.tensor.matmul(
                    out=pg[:], lhsT=xT_s[:, k * P : (k + 1) * P],
                    rhs=rhs1[:], start=True, stop=True,
                )
                if k == 0:
                    # a_dst per partition from the self-loop rows, then fold
                    # in the validity mask for every chunk at once
                    nc.vector.tensor_copy(ad_part[:], pg[:, :H1])
                    nc.vector.tensor_tensor(
                        out=admix[:].rearrange("p (k h) -> p k h", h=H1),
                        in0=ad_part[:].rearrange("p (o h) -> p o h", o=1)
                        .to_broadcast([P, K, H1]),
                        in1=fsl("admask")[:].rearrange("p (k o) -> p k o", o=1)
                        .to_broadcast([P, K, H1]),
                        op=ADD,
                    )
                e = wp.tile([P, H1], f32, tag="e")
                nc.vector.tensor_tensor(
                    out=e[:], in0=pg[:, H1 : 2 * H1],
                    in1=admix[:, k * H1 : (k + 1) * H1], op=ADD,
                )
                es = wp.tile([P, H1], f32, tag="es")
                nc.vector.tensor_scalar_mul(es[:], e[:], 0.2)
                el = wp.tile([P, H1], f32, tag="el")
                nc.vector.tensor_tensor(out=el[:], in0=e[:], in1=es[:], op=MAX)
                pe = wp.tile([P, H1], f32, tag="pe")
                nc.scalar.activation(pe[:], el[:], Exp)
                msg = wp.tile([P, D1], f32, tag="msg")
                nc.vector.tensor_tensor(
                    out=msg[:].rearrange("p (h c) -> p h c", c=C),
                    in0=pg[:, 2 * H1 :].rearrange("p (h c) -> p h c", c=C),
                    in1=pe[:].rearrange("p (h o) -> p h o", o=1).to_broadcast(
                        [P, H1, C]
                    ),
                    op=MUL,
                )
                nc.gpsimd.tensor_tensor(
                    out=acc[:, :D1], in0=acc[:, :D1], in1=msg[:], op=ADD
                )
                nc.vector.tensor_tensor(
                    out=acc[:, D1:], in0=acc[:, D1:], in1=pe[:], op=ADD
                )

            # merge partition groups per destination: one-hot matmul
            pmap = fp.tile([P, P], f32)
            nc.vector.tensor_scalar(
                pmap[:], iota_f[:], fsl("pmapcol")[:, 0:1], None, EQ
            )
            agg1 = ap_.tile([P, D1 + H1], f32, tag="agg1")
            nc.tensor.matmul(out=agg1[:], lhsT=pmap[:], rhs=acc[:],
                             start=True, stop=True)

            # ---------------- layer-1 finalize: h1r = relu(num/den + b1) ---
            den1 = fp.tile([P, H1], f32)
            nc.vector.tensor_scalar_add(den1[:], agg1[:, D1:], 1e-16)
            rec1 = fp.tile([P, H1], f32)
            nc.vector.reciprocal(rec1[:], den1[:])
            h1t = fp.tile([P, D1], f32)
            nc.vector.tensor_tensor(
                out=h1t[:].rearrange("p (h c) -> p h c", c=C),
                in0=agg1[:, :D1].rearrange("p (h c) -> p h c", c=C),
                in1=rec1[:].rearrange("p (h o) -> p h o", o=1).to_broadcast(
                    [P, H1, C]
                ),
                op=MUL,
            )
            h1b = fp.tile([P, D1], f32)
            nc.vector.tensor_add(h1b[:], h1t[:], fsl("b1bc"))
            h1r = fp.tile([P, D1], f32)
            nc.scalar.activation(h1r[:], h1b[:], Relu)

            # ---------------- layer-2 projection: G2 = [ad2|as2|h2p] -------
            # rhs per K-chunk (independent of h1r, scheduled early)
            rhs2 = []
            for k in range(KCH):
                pwa2 = pp.tile([P, 2], f32, tag="mm")
                nc.tensor.matmul(
                    out=pwa2[:], lhsT=fsl("w2T", C)[:, k * P : (k + 1) * P],
                    rhs=fsl("a2", C), start=True, stop=True,
                )
                rhs2_k = cp.tile([P, G2W], f32, name=f"rhs2_{k}")
                nc.vector.tensor_copy(rhs2_k[:, :2], pwa2[:])
                nc.scalar.copy(rhs2_k[:, 2:], fsl(f"w2_{k}"))
                rhs2.append(rhs2_k)
            # layer-2 one-hots from the replicated [src2 | dstrel2] rows
            l2r = fsl("l2rep")
            st2 = fp.tile([P, P], bf16)
            nc.vector.tensor_scalar(st2[:], l2r[:, :P], iota_c[:, 0:1], None, EQ)
            em2t = fp.tile([P, P], f32)
            nc.vector.tensor_scalar(em2t[:], l2r[:, P:], iota_c[:, 0:1], None, EQ)

            pg2 = ap_.tile([P, G2W], f32, tag="pg2")
            for k in range(KCH):
                ptr = pp.tile([P, P], f32, tag="mm")
                nc.tensor.transpose(
                    out=ptr[:], in_=h1r[:, k * P : (k + 1) * P], identity=ident[:]
                )
                h1rT_k = wp.tile([P, P], f32, tag=f"h1rTk{k}")
                nc.vector.tensor_copy(h1rT_k[:], ptr[:])
                nc.tensor.matmul(
                    out=pg2[:], lhsT=h1rT_k[:], rhs=rhs2[k][:],
                    start=(k == 0), stop=(k == KCH - 1),
                )
            g2sb = fp.tile([P, G2W], bf16)
            nc.scalar.copy(g2sb[:], pg2[:])
            g2ad = fp.tile([P, 1], f32)
            nc.vector.tensor_copy(g2ad[:], pg2[:, 0:1])

            # ------- layer-2 edge aggregation: fully on-chip (ball only) ---
            gs2_p = pp.tile([P, G2W], f32, tag="mm")
            nc.tensor.matmul(out=gs2_p[:], lhsT=st2[:], rhs=g2sb[:],
                             start=True, stop=False, skip_group_check=True)
            # accumulate the a_dst[ball] expansion straight onto the as2
            # column: gs2_p[:, 1] becomes e2 = as2[src] + ad2[dst]
            nc.tensor.matmul(out=gs2_p[:, 1:2], lhsT=em2t[:], rhs=g2ad[:],
                             start=False, stop=True, skip_group_check=True)
            es2 = fp.tile([P, 1], f32)
            nc.vector.tensor_scalar_mul(es2[:], gs2_p[:, 1:2], 0.2)
            el2 = fp.tile([P, 1], f32)
            nc.vector.tensor_tensor(
                out=el2[:], in0=gs2_p[:, 1:2], in1=es2[:], op=MAX
            )
            pe2 = fp.tile([P, 1], f32)
            nc.scalar.activation(pe2[:], el2[:], Exp)
            rhs2t = fp.tile([P, C + 1], bf16)
            nc.vector.tensor_tensor(
                out=rhs2t[:, :C], in0=gs2_p[:, 2:],
                in1=pe2[:].to_broadcast([P, C]), op=MUL,
            )
            nc.scalar.activation(rhs2t[:, C:], pe2[:], Copy)
            em2 = fp.tile([P, P], bf16)
            nc.vector.tensor_scalar(
                em2[:], iota_f[:], fsl("dstrel2")[:, 0:1], None, EQ
            )
            agg2 = ap_.tile([P, C + 1], f32, tag="agg2")
            nc.tensor.matmul(out=agg2[:], lhsT=em2[:], rhs=rhs2t[:],
                             start=True, stop=True)

            # ---------------- ball finalize + MLP --------------------------
            den2 = fp.tile([1, 1], f32)
            nc.vector.tensor_scalar_add(den2[:], agg2[0:1, C : C + 1], 1e-16)
            rec2 = fp.tile([1, 1], f32)
            nc.vector.reciprocal(rec2[:], den2[:])
            bf = fp.tile([1, C], f32)
            nc.scalar.activation(bf[:], agg2[0:1, :C], Copy, scale=rec2[:, 0:1])
            ptb = pp.tile([C, 1], f32, tag="mm")
            nc.tensor.transpose(out=ptb[:], in_=bf[:], identity=ident[0:1, 0:1])
            bfr = fp.tile([C, 1], f32)
            nc.scalar.activation(bfr[:], ptb[:], Relu, bias=fsl("b2col", C))

            pz = pp.tile([C // 2, 1], f32, tag="mm")
            nc.tensor.matmul(out=pz[:], lhsT=fsl("fc1w", C), rhs=bfr[:],
                             start=True, stop=True)
            zr = fp.tile([C // 2, 1], f32)
            nc.scalar.activation(zr[:], pz[:], Relu, bias=fsl("fc1b", C // 2))

            po = pp.tile([2, 1], f32, tag="mm")
            nc.tensor.matmul(out=po[:], lhsT=fsl("fc2w", C // 2), rhs=zr[:],
                             start=True, stop=True)
            osb = fp.tile([2, 1], f32)
            nc.vector.tensor_add(osb[:], po[:], fsl("fc2b", 2))
            nc.sync.dma_start(out_d[:], osb[:])

    nc.compile()
    return nc


def kernel(**inputs):
    from concourse.bass_utils import run_bass_kernel_spmd

    feed, dims = _host_preprocess(inputs)
    key = (dims["K"], dims["m2"], dims["T2"])
    if key not in _CACHE:
        _CACHE[key] = _build(dims)
    nc = _CACHE[key]

    n_cores = 8
    in_maps = [dict(feed) for _ in range(n_cores)]
    res = run_bass_kernel_spmd(nc, in_maps, core_ids=list(range(n_cores)))
    out = np.asarray(res.results[0]["out"], dtype=np.float32).reshape(2)
    return out


# revision 15
# speedup vs baseline: 2.4373x; 1.1306x over previous
"""Trainium2 Bass kernel for nn_BallPredictorGNN.

The reference model is a 2-layer GAT over (N=20000, E=640000) followed by an
MLP applied to the LAST node only ("ball") — the output is a single [2] vector.
Only the ball's 2-hop dependency cone matters:

  layer 2 aggregates at the ball node only            (~25 in-edges)
  layer 1 aggregates at the ball's in-neighbours S2   (~25 nodes, ~800 edges)
  x @ W1 is needed for the sources of those edges     (~800 edges)

Host side (pure data routing): extract the cone and lay layer-1 edges out on a
[128 partitions x K chunks] grid, where each partition serves one destination
node (high-degree destinations get several partitions, merged at the end by a
single one-hot matmul).  The source features are replicated per edge-slot into
the xT operand, so the projection matmul directly produces per-edge rows
[ad | as | h] = x_src @ [W1Ad | W1As | W1] in the right partition — no
gather, no DRAM round-trip, no indirect DMA anywhere.

Device side (all FLOPs): per chunk, one TensorE matmul projects 128 edges;
VectorE computes e = as + ad + mask, leaky-relu, and accumulates
msg += h * exp(e) and den += exp(e) along the free axis
(alpha = exp(e)/sum exp(e) folded as  out = (sum exp(e)*h_src) / sum exp(e);
masked/padded slots contribute exp(-1e30) = 0).  Layer 2 (ball only) runs
fully on-chip with one-hot matmuls against the SBUF-resident projection.

The same program is replicated SPMD on all 8 NeuronCores (the cone is tiny, so
replication beats sharding + collectives); core 0's output is returned.
"""

import numpy as np

P = 128
_CACHE = {}


def _ceil(a, b):
    return -(-a // b)


def _pad_rows(a, n, fill=0):
    out = np.full((n,) + a.shape[1:], fill, a.dtype)
    out[: len(a)] = a
    return out


class _Packer:
    """Pack many small [p, w] operands into one [128, W] array, column-wise."""

    def __init__(self, dtype=np.float32):
        self.cols = []
        self.pos = 0
        self.slots = {}
        self.dtype = dtype

    def add(self, name, arr):
        p, w = arr.shape
        full = np.zeros((P, w), self.dtype)
        full[:p] = arr
        self.cols.append(full)
        self.slots[name] = (self.pos, self.pos + w)
        self.pos += w

    def finish(self):
        return np.ascontiguousarray(np.concatenate(self.cols, axis=1))


NEG = np.float32(-1e30)


def _host_preprocess(inputs):
    x = np.asarray(inputs["x"], dtype=np.float32)
    ei = np.asarray(inputs["edge_index"]).astype(np.int64)
    N = x.shape[0]
    F = x.shape[1]
    ball = N - 1
    src, dst = ei[0], ei[1]

    # ---- layer-2 cone: edges into the ball (+ self loop) --------------------
    sel2 = dst == ball
    e2s = np.concatenate([src[sel2], [ball]])
    uniq = np.unique(e2s)
    S2 = np.concatenate([[ball], uniq[uniq != ball]]).astype(np.int64)
    m2 = len(S2)
    assert m2 <= 127, f"ball in-neighbourhood too large for one dst block: {m2}"

    # ---- layer-1 edge grid: [partition, chunk] ------------------------------
    in_S2 = np.zeros(N, dtype=bool)
    in_S2[S2] = True
    sel1 = in_S2[dst]
    l1s, l1d = src[sel1], dst[sel1]  # self loops handled separately

    # per-destination source lists (excluding the self loop)
    loc2 = np.full(N, -1, dtype=np.int64)
    loc2[S2] = np.arange(m2)
    by_dst = [[] for _ in range(m2)]
    for s, d in zip(l1s, loc2[l1d]):
        by_dst[d].append(s)

    # choose K (chunks) so all partition groups fit in 128 partitions
    K = 2
    while sum(max(1, _ceil(len(g), K - 1)) for g in by_dst) > P:
        K += 1
    K = max(K, 3)

    grid_src = np.zeros((P, K), dtype=np.int64)  # source node per slot
    grid_valid = np.zeros((P, K), dtype=bool)
    slotmap = np.full(P, P - 1, dtype=np.int64)  # partition -> dst slot
    p = 0
    for sidx in range(m2):
        g = by_dst[sidx]
        v = S2[sidx]
        nparts = max(1, _ceil(len(g), K - 1))
        for gi in range(nparts):
            grid_src[p, 0] = v  # self loop (duplicates masked)
            grid_valid[p, 0] = gi == 0
            chunk_edges = g[gi * (K - 1) : (gi + 1) * (K - 1)]
            for j, s in enumerate(chunk_edges):
                grid_src[p, 1 + j] = s
                grid_valid[p, 1 + j] = True
            slotmap[p] = sidx
            p += 1
    assert p <= P

    # xT: [F, K*128] with column k*128+q = x[grid_src[q, k]] (bf16).
    # Chunk-0 columns keep their features even when masked: secondary
    # partitions read a_dst[dst] from their (duplicate) self-loop row.
    zero_slots = ~grid_valid
    zero_slots[:, 0] = False
    xg = x[grid_src.T.reshape(-1)]  # [K*128, F]
    xg[zero_slots.T.reshape(-1)] = 0
    import ml_dtypes

    xT = np.ascontiguousarray(xg.T.astype(ml_dtypes.bfloat16))  # [F, K*128]

    admask = np.where(grid_valid, np.float32(0), NEG).astype(np.float32)  # [P,K]
    pmapcol = slotmap[:, None].astype(np.float32)  # [P,1]

    # ---- layer-2 index rows -------------------------------------------------
    s2_loc = loc2[e2s]  # all < m2
    n2 = len(s2_loc)
    T2 = _ceil(n2, P)
    assert T2 == 1, f"layer-2 edge count exceeds one tile: {n2}"
    n2p = T2 * P

    def row(a, n_pad, fill, dt):
        return _pad_rows(a.astype(dt), n_pad, fill)[None, :]

    # ---- dense operands -----------------------------------------------------
    W1 = np.asarray(inputs["W1"], np.float32)  # [F, 4*64]
    a_src1 = np.asarray(inputs["a_src1"], np.float32)  # [4, 64]
    a_dst1 = np.asarray(inputs["a_dst1"], np.float32)
    H1, C = a_src1.shape
    D1 = H1 * C
    ablk = np.zeros((D1, 2 * H1), np.float32)  # [256, 8] = [Ad | As]
    for h in range(H1):
        ablk[h * C : (h + 1) * C, h] = a_dst1[h]
        ablk[h * C : (h + 1) * C, H1 + h] = a_src1[h]

    W2 = np.asarray(inputs["W2"], np.float32)  # [256, 64]
    a2 = np.stack(
        [np.asarray(inputs["a_dst2"], np.float32)[0],
         np.asarray(inputs["a_src2"], np.float32)[0]],
        axis=1,
    )  # [64, 2] = [a_dst | a_src]

    pkf = _Packer()
    pkf.add("w1", W1)
    W1T = np.ascontiguousarray(W1.T)
    for k in range(D1 // P):
        pkf.add(f"w1T{k}", W1T[k * P : (k + 1) * P])
        pkf.add(f"ablk{k}", ablk[k * P : (k + 1) * P])
        pkf.add(f"w2_{k}", W2[k * P : (k + 1) * P])
    pkf.add("b1bc", np.broadcast_to(np.asarray(inputs["b1"], np.float32), (P, D1)))
    pkf.add("w2T", np.ascontiguousarray(W2.T))
    pkf.add("a2", a2)
    pkf.add("b2col", np.asarray(inputs["b2"], np.float32)[:, None])
    pkf.add("fc1w", np.ascontiguousarray(np.asarray(inputs["fc1_w"], np.float32)))
    pkf.add("fc1b", np.asarray(inputs["fc1_b"], np.float32)[:, None])
    pkf.add("fc2w", np.ascontiguousarray(np.asarray(inputs["fc2_w"], np.float32)))
    pkf.add("fc2b", np.asarray(inputs["fc2_b"], np.float32)[:, None])
    pkf.add("admask", admask)
    pkf.add("pmapcol", pmapcol)
    pkf.add("dstrel2", np.ascontiguousarray(
        _pad_rows(np.zeros(n2, np.float32), n2p, 1)[:, None]))
    pkf.add("l2rep", np.broadcast_to(np.concatenate(
        [row(s2_loc, n2p, 0, np.float32),
         row(np.zeros(n2), n2p, 1, np.float32)], axis=1), (P, 2 * n2p)))

    feed = {"xT": xT, "packf": pkf.finish()}
    dims = dict(
        F=F, H1=H1, C=C, K=K, m2=m2, T2=T2,
        slots_f=tuple(sorted(pkf.slots.items())),
    )
    return feed, dims


def _build(dims):
    from concourse import bacc, bass, mybir, tile
    from concourse.masks import make_identity

    F = dims["F"]          # 128 input features
    H1 = dims["H1"]        # 4 heads, layer 1
    C = dims["C"]          # 64 channels per head
    D1 = H1 * C            # 256
    G1W = 2 * H1 + D1      # 264 = [ad(4) | as(4) | h(256)]
    G2W = 2 + C            # 66  = [ad2 | as2 | h2p]
    K = dims["K"]          # layer-1 chunks (edge slots per partition)
    KCH = D1 // P          # 2 contraction chunks over 256
    slots_f = dict(dims["slots_f"])
    WF = max(b for _, b in slots_f.values())
    f32 = mybir.dt.float32
    bf16 = mybir.dt.bfloat16

    nc = bacc.Bacc("TRN2", target_bir_lowering=False, debug=False)

    xT_d = nc.declare_dram_parameter("xT", [F, K * P], bf16, isOutput=False)
    pf_d = nc.declare_dram_parameter("packf", [P, WF], f32, isOutput=False)
    out_d = nc.declare_dram_parameter("out", [2, 1], f32, isOutput=True)

    EQ = mybir.AluOpType.is_equal
    MAX = mybir.AluOpType.max
    ADD = mybir.AluOpType.add
    MUL = mybir.AluOpType.mult
    Copy = mybir.ActivationFunctionType.Copy
    Exp = mybir.ActivationFunctionType.Exp
    Relu = mybir.ActivationFunctionType.Relu

    with tile.TileContext(nc) as tc:
        with (
            tc.tile_pool(name="const", bufs=1) as cp,
            tc.tile_pool(name="work", bufs=3) as wp,
            tc.tile_pool(name="fin", bufs=1) as fp,
            tc.tile_pool(name="psum", bufs=2, space="PSUM") as pp,
            tc.tile_pool(name="pgp", bufs=3, space="PSUM") as pgp,
            tc.tile_pool(name="acc", bufs=1, space="PSUM") as ap_,
        ):
            # ---------------- inputs into SBUF -----------------------------
            pk = cp.tile([P, WF], f32)
            nc.sync.dma_start(pk[:], pf_d[:])
            xT_s = cp.tile([F, K * P], bf16)
            for ci in range(K):
                eng = nc.sync if ci % 2 == 0 else nc.gpsimd
                eng.dma_start(
                    xT_s[:, ci * P : (ci + 1) * P],
                    xT_d[:, ci * P : (ci + 1) * P],
                )

            def fsl(name, rows=P):
                a, b = slots_f[name]
                return pk[:rows, a:b]

            ident = cp.tile([P, P], f32)
            make_identity(nc, ident[:])
            iota_f = cp.tile([P, P], f32)
            nc.gpsimd.iota(
                iota_f[:], pattern=[[1, P]], base=0, channel_multiplier=0,
                allow_small_or_imprecise_dtypes=True,
            )
            iota_c = cp.tile([P, 1], f32)
            nc.gpsimd.iota(
                iota_c[:], pattern=[[0, 1]], base=0, channel_multiplier=1,
                allow_small_or_imprecise_dtypes=True,
            )

            # ---------------- W1 @ [Ad | As]  (K = 256, 2 chunks) ----------
            pwa = pp.tile([F, 2 * H1], f32, tag="mm")
            for k in range(KCH):
                nc.tensor.matmul(
                    out=pwa[:], lhsT=fsl(f"w1T{k}"), rhs=fsl(f"ablk{k}"),
                    start=(k == 0), stop=(k == KCH - 1),
                )
            rhs1 = cp.tile([F, G1W], bf16)
            nc.vector.tensor_copy(rhs1[:, : 2 * H1], pwa[:])
            nc.scalar.copy(rhs1[:, 2 * H1 :], fsl("w1"))

            # ---------------- layer-1 edge chunks --------------------------
            # chunk k: project 128 edge slots, form [h*exp(e) | exp(e)], and
            # accumulate + merge partition groups in PSUM via the one-hot
            # partition->slot matmul:  agg1 += pmap @ [msg | exp(e)]
            pmap = fp.tile([P, P], bf16)
            nc.vector.tensor_scalar(
                pmap[:], iota_f[:], fsl("pmapcol")[:, 0:1], None, EQ
            )
            agg1 = ap_.tile([P, D1 + H1], f32, tag="agg1")
            ad_part = fp.tile([P, H1], f32)
            admix = fp.tile([P, K * H1], f32)
            for k in range(K):
                pg = pgp.tile([P, G1W], f32, tag="pg")
                nc.tensor.matmul(
                    out=pg[:], lhsT=xT_s[:, k * P : (k + 1) * P],
                    rhs=rhs1[:], start=True, stop=True,
                )
                if k == 0:
                    # a_dst per partition from the self-loop rows, then fold
                    # in the validity mask for every chunk at once
                    nc.vector.tensor_copy(ad_part[:], pg[:, :H1])
                    nc.vector.tensor_tensor(
                        out=admix[:].rearrange("p (k h) -> p k h", h=H1),
                        in0=ad_part[:].rearrange("p (o h) -> p o h", o=1)
                        .to_broadcast([P, K, H1]),
                        in1=fsl("admask")[:].rearrange("p (k o) -> p k o", o=1)
                        .to_broadcast([P, K, H1]),
                        op=ADD,
                    )
                e = wp.tile([P, H1], f32, tag="e")
                nc.vector.tensor_tensor(
                    out=e[:], in0=pg[:, H1 : 2 * H1],
                    in1=admix[:, k * H1 : (k + 1) * H1], op=ADD,
                )
                es = wp.tile([P, H1], f32, tag="es")
                nc.vector.tensor_scalar_mul(es[:], e[:], 0.2)
                el = wp.tile([P, H1], f32, tag="el")
                nc.vector.tensor_tensor(out=el[:], in0=e[:], in1=es[:], op=MAX)
                pe = wp.tile([P, H1], f32, tag="pe")
                nc.scalar.activation(pe[:], el[:], Exp)
                msg = wp.tile([P, D1 + H1], bf16, tag="msg")
                nc.vector.tensor_tensor(
                    out=msg[:, :D1].rearrange("p (h c) -> p h c", c=C),
                    in0=pg[:, 2 * H1 :].rearrange("p (h c) -> p h c", c=C),
                    in1=pe[:].rearrange("p (h o) -> p h o", o=1).to_broadcast(
                        [P, H1, C]
                    ),
                    op=MUL,
                )
                nc.scalar.activation(msg[:, D1:], pe[:], Copy)
                nc.tensor.matmul(
                    out=agg1[:], lhsT=pmap[:], rhs=msg[:],
                    start=(k == 0), stop=(k == K - 1),
                )

            # ---------------- layer-1 finalize: h1r = relu(num/den + b1) ---
            den1 = fp.tile([P, H1], f32)
            nc.vector.tensor_scalar_add(den1[:], agg1[:, D1:], 1e-16)
            rec1 = fp.tile([P, H1], f32)
            nc.vector.reciprocal(rec1[:], den1[:])
            h1t = fp.tile([P, D1], f32)
            nc.vector.tensor_tensor(
                out=h1t[:].rearrange("p (h c) -> p h c", c=C),
                in0=agg1[:, :D1].rearrange("p (h c) -> p h c", c=C),
                in1=rec1[:].rearrange("p (h o) -> p h o", o=1).to_broadcast(
                    [P, H1, C]
                ),
                op=MUL,
            )
            h1b = fp.tile([P, D1], f32)
            nc.vector.tensor_add(h1b[:], h1t[:], fsl("b1bc"))
            h1r = fp.tile([P, D1], f32)
            nc.scalar.activation(h1r[:], h1b[:], Relu)

            # ---------------- layer-2 projection: G2 = [ad2|as2|h2p] -------
            # rhs per K-chunk (independent of h1r, scheduled early)
            rhs2 = []
            for k in range(KCH):
                pwa2 = pp.tile([P, 2], f32, tag="mm")
                nc.tensor.matmul(
                    out=pwa2[:], lhsT=fsl("w2T", C)[:, k * P : (k + 1) * P],
                    rhs=fsl("a2", C), start=True, stop=True,
                )
                rhs2_k = cp.tile([P, G2W], f32, name=f"rhs2_{k}")
                nc.vector.tensor_copy(rhs2_k[:, :2], pwa2[:])
                nc.scalar.copy(rhs2_k[:, 2:], fsl(f"w2_{k}"))
                rhs2.append(rhs2_k)
            # layer-2 one-hots from the replicated [src2 | dstrel2] rows
            l2r = fsl("l2rep")
            st2 = fp.tile([P, P], bf16)
            nc.vector.tensor_scalar(st2[:], l2r[:, :P], iota_c[:, 0:1], None, EQ)
            em2t = fp.tile([P, P], f32)
            nc.vector.tensor_scalar(em2t[:], l2r[:, P:], iota_c[:, 0:1], None, EQ)

            pg2 = ap_.tile([P, G2W], f32, tag="pg2")
            for k in range(KCH):
                ptr = pp.tile([P, P], f32, tag="mm")
                nc.tensor.transpose(
                    out=ptr[:], in_=h1r[:, k * P : (k + 1) * P], identity=ident[:]
                )
                h1rT_k = wp.tile([P, P], f32, tag=f"h1rTk{k}")
                nc.vector.tensor_copy(h1rT_k[:], ptr[:])
                nc.tensor.matmul(
                    out=pg2[:], lhsT=h1rT_k[:], rhs=rhs2[k][:],
                    start=(k == 0), stop=(k == KCH - 1),
                )
            g2sb = fp.tile([P, G2W], bf16)
            nc.scalar.copy(g2sb[:], pg2[:])
            g2ad = fp.tile([P, 1], f32)
            nc.vector.tensor_copy(g2ad[:], pg2[:, 0:1])

            # ------- layer-2 edge aggregation: fully on-chip (ball only) ---
            gs2_p = pp.tile([P, G2W], f32, tag="mm")
            nc.tensor.matmul(out=gs2_p[:], lhsT=st2[:], rhs=g2sb[:],
                             start=True, stop=False, skip_group_check=True)
            # accumulate the a_dst[ball] expansion straight onto the as2
            # column: gs2_p[:, 1] becomes e2 = as2[src] + ad2[dst]
            nc.tensor.matmul(out=gs2_p[:, 1:2], lhsT=em2t[:], rhs=g2ad[:],
                             start=False, stop=True, skip_group_check=True)
            es2 = fp.tile([P, 1], f32)
            nc.vector.tensor_scalar_mul(es2[:], gs2_p[:, 1:2], 0.2)
            el2 = fp.tile([P, 1], f32)
            nc.vector.tensor_tensor(
                out=el2[:], in0=gs2_p[:, 1:2], in1=es2[:], op=MAX
            )
            pe2 = fp.tile([P, 1], f32)
            nc.scalar.activation(pe2[:], el2[:], Exp)
            rhs2t = fp.tile([P, C + 1], bf16)
            nc.vector.tensor_tensor(
                out=rhs2t[:, :C], in0=gs2_p[:, 2:],
                in1=pe2[:].to_broadcast([P, C]), op=MUL,
            )
            nc.scalar.activation(rhs2t[:, C:], pe2[:], Copy)
            em2 = fp.tile([P, P], bf16)
            nc.vector.tensor_scalar(
                em2[:], iota_f[:], fsl("dstrel2")[:, 0:1], None, EQ
            )
            agg2 = ap_.tile([P, C + 1], f32, tag="agg2")
            nc.tensor.matmul(out=agg2[:], lhsT=em2[:], rhs=rhs2t[:],
                             start=True, stop=True)

            # ---------------- ball finalize + MLP --------------------------
            den2 = fp.tile([1, 1], f32)
            nc.vector.tensor_scalar_add(den2[:], agg2[0:1, C : C + 1], 1e-16)
            rec2 = fp.tile([1, 1], f32)
            nc.vector.reciprocal(rec2[:], den2[:])
            bf = fp.tile([1, C], f32)
            nc.scalar.activation(bf[:], agg2[0:1, :C], Copy, scale=rec2[:, 0:1])
            ptb = pp.tile([C, 1], f32, tag="mm")
            nc.tensor.transpose(out=ptb[:], in_=bf[:], identity=ident[0:1, 0:1])
            bfr = fp.tile([C, 1], f32)
            nc.scalar.activation(bfr[:], ptb[:], Relu, bias=fsl("b2col", C))

            pz = pp.tile([C // 2, 1], f32, tag="mm")
            nc.tensor.matmul(out=pz[:], lhsT=fsl("fc1w", C), rhs=bfr[:],
                             start=True, stop=True)
            zr = fp.tile([C // 2, 1], f32)
            nc.scalar.activation(zr[:], pz[:], Relu, bias=fsl("fc1b", C // 2))

            po = pp.tile([2, 1], f32, tag="mm")
            nc.tensor.matmul(out=po[:], lhsT=fsl("fc2w", C // 2), rhs=zr[:],
                             start=True, stop=True)
            osb = fp.tile([2, 1], f32)
            nc.vector.tensor_add(osb[:], po[:], fsl("fc2b", 2))
            nc.sync.dma_start(out_d[:], osb[:])

    nc.compile()
    return nc


def kernel(**inputs):
    from concourse.bass_utils import run_bass_kernel_spmd

    feed, dims = _host_preprocess(inputs)
    key = (dims["K"], dims["m2"], dims["T2"])
    if key not in _CACHE:
        _CACHE[key] = _build(dims)
    nc = _CACHE[key]

    n_cores = 8
    in_maps = [dict(feed) for _ in range(n_cores)]
    res = run_bass_kernel_spmd(nc, in_maps, core_ids=list(range(n_cores)))
    out = np.asarray(res.results[0]["out"], dtype=np.float32).reshape(2)
    return out


# revision 16
# speedup vs baseline: 2.5302x; 1.0381x over previous
"""Trainium2 Bass kernel for nn_BallPredictorGNN.

The reference model is a 2-layer GAT over (N=20000, E=640000) followed by an
MLP applied to the LAST node only ("ball") — the output is a single [2] vector.
Only the ball's 2-hop dependency cone matters:

  layer 2 aggregates at the ball node only            (~25 in-edges)
  layer 1 aggregates at the ball's in-neighbours S2   (~25 nodes, ~800 edges)
  x @ W1 is needed for the sources of those edges     (~800 edges)

Host side (pure data routing): extract the cone and lay layer-1 edges out on a
[128 partitions x K chunks] grid, where each partition serves one destination
node (high-degree destinations get several partitions, merged at the end by a
single one-hot matmul).  The source features are replicated per edge-slot into
the xT operand, so the projection matmul directly produces per-edge rows
[ad | as | h] = x_src @ [W1Ad | W1As | W1] in the right partition — no
gather, no DRAM round-trip, no indirect DMA anywhere.

Device side (all FLOPs): per chunk, one TensorE matmul projects 128 edges;
VectorE computes e = as + ad + mask, leaky-relu, and accumulates
msg += h * exp(e) and den += exp(e) along the free axis
(alpha = exp(e)/sum exp(e) folded as  out = (sum exp(e)*h_src) / sum exp(e);
masked/padded slots contribute exp(-1e30) = 0).  Layer 2 (ball only) runs
fully on-chip with one-hot matmuls against the SBUF-resident projection.

The same program is replicated SPMD on all 8 NeuronCores (the cone is tiny, so
replication beats sharding + collectives); core 0's output is returned.
"""

import numpy as np

P = 128
_CACHE = {}


def _ceil(a, b):
    return -(-a // b)


def _pad_rows(a, n, fill=0):
    out = np.full((n,) + a.shape[1:], fill, a.dtype)
    out[: len(a)] = a
    return out


class _Packer:
    """Pack many small [p, w] operands into one [128, W] array, column-wise."""

    def __init__(self, dtype=np.float32):
        self.cols = []
        self.pos = 0
        self.slots = {}
        self.dtype = dtype

    def add(self, name, arr):
        p, w = arr.shape
        full = np.zeros((P, w), self.dtype)
        full[:p] = arr
        self.cols.append(full)
        self.slots[name] = (self.pos, self.pos + w)
        self.pos += w

    def finish(self):
        return np.ascontiguousarray(np.concatenate(self.cols, axis=1))


NEG = np.float32(-1e30)


def _host_preprocess(inputs):
    x = np.asarray(inputs["x"], dtype=np.float32)
    ei = np.asarray(inputs["edge_index"]).astype(np.int64)
    N = x.shape[0]
    F = x.shape[1]
    ball = N - 1
    src, dst = ei[0], ei[1]

    # ---- layer-2 cone: edges into the ball (+ self loop) --------------------
    sel2 = dst == ball
    e2s = np.concatenate([src[sel2], [ball]])
    uniq = np.unique(e2s)
    S2 = np.concatenate([[ball], uniq[uniq != ball]]).astype(np.int64)
    m2 = len(S2)
    assert m2 <= 127, f"ball in-neighbourhood too large for one dst block: {m2}"

    # ---- layer-1 edge grid: [partition, chunk] ------------------------------
    in_S2 = np.zeros(N, dtype=bool)
    in_S2[S2] = True
    sel1 = in_S2[dst]
    l1s, l1d = src[sel1], dst[sel1]  # self loops handled separately

    # per-destination source lists (excluding the self loop)
    loc2 = np.full(N, -1, dtype=np.int64)
    loc2[S2] = np.arange(m2)
    by_dst = [[] for _ in range(m2)]
    for s, d in zip(l1s, loc2[l1d]):
        by_dst[d].append(s)

    # choose K (chunks) so all partition groups fit in 128 partitions
    K = 2
    while sum(max(1, _ceil(len(g), K - 1)) for g in by_dst) > P:
        K += 1
    K = max(K, 3)

    grid_src = np.zeros((P, K), dtype=np.int64)  # source node per slot
    grid_valid = np.zeros((P, K), dtype=bool)
    slotmap = np.full(P, P - 1, dtype=np.int64)  # partition -> dst slot
    p = 0
    for sidx in range(m2):
        g = by_dst[sidx]
        v = S2[sidx]
        nparts = max(1, _ceil(len(g), K - 1))
        for gi in range(nparts):
            grid_src[p, 0] = v  # self loop (duplicates masked)
            grid_valid[p, 0] = gi == 0
            chunk_edges = g[gi * (K - 1) : (gi + 1) * (K - 1)]
            for j, s in enumerate(chunk_edges):
                grid_src[p, 1 + j] = s
                grid_valid[p, 1 + j] = True
            slotmap[p] = sidx
            p += 1
    assert p <= P

    # xT: [F, K*128] with column k*128+q = x[grid_src[q, k]] (bf16).
    # Chunk-0 columns keep their features even when masked: secondary
    # partitions read a_dst[dst] from their (duplicate) self-loop row.
    zero_slots = ~grid_valid
    zero_slots[:, 0] = False
    xg = x[grid_src.T.reshape(-1)]  # [K*128, F]
    xg[zero_slots.T.reshape(-1)] = 0
    import ml_dtypes

    xT = np.ascontiguousarray(xg.T.astype(ml_dtypes.bfloat16))  # [F, K*128]

    admask = np.where(grid_valid, np.float32(0), NEG).astype(np.float32)  # [P,K]
    pmapcol = slotmap[:, None].astype(np.float32)  # [P,1]

    # ---- layer-2 index rows -------------------------------------------------
    s2_loc = loc2[e2s]  # all < m2
    n2 = len(s2_loc)
    T2 = _ceil(n2, P)
    assert T2 == 1, f"layer-2 edge count exceeds one tile: {n2}"
    n2p = T2 * P

    def row(a, n_pad, fill, dt):
        return _pad_rows(a.astype(dt), n_pad, fill)[None, :]

    # ---- dense operands -----------------------------------------------------
    W1 = np.asarray(inputs["W1"], np.float32)  # [F, 4*64]
    a_src1 = np.asarray(inputs["a_src1"], np.float32)  # [4, 64]
    a_dst1 = np.asarray(inputs["a_dst1"], np.float32)
    H1, C = a_src1.shape
    D1 = H1 * C
    ablk = np.zeros((D1, 2 * H1), np.float32)  # [256, 8] = [Ad | As]
    for h in range(H1):
        ablk[h * C : (h + 1) * C, h] = a_dst1[h]
        ablk[h * C : (h + 1) * C, H1 + h] = a_src1[h]

    W2 = np.asarray(inputs["W2"], np.float32)  # [256, 64]
    a2 = np.stack(
        [np.asarray(inputs["a_dst2"], np.float32)[0],
         np.asarray(inputs["a_src2"], np.float32)[0]],
        axis=1,
    )  # [64, 2] = [a_dst | a_src]

    pka = _Packer()
    pka.add("w1", W1)
    W1T = np.ascontiguousarray(W1.T)
    for k in range(D1 // P):
        pka.add(f"w1T{k}", W1T[k * P : (k + 1) * P])
        pka.add(f"ablk{k}", ablk[k * P : (k + 1) * P])
    pka.add("admask", admask)
    pka.add("pmapcol", pmapcol)

    pkb = _Packer()
    for k in range(D1 // P):
        pkb.add(f"w2_{k}", W2[k * P : (k + 1) * P])
    pkb.add("b1bc", np.broadcast_to(np.asarray(inputs["b1"], np.float32), (P, D1)))
    pkb.add("w2T", np.ascontiguousarray(W2.T))
    pkb.add("a2", a2)
    pkb.add("b2col", np.asarray(inputs["b2"], np.float32)[:, None])
    pkb.add("fc1w", np.ascontiguousarray(np.asarray(inputs["fc1_w"], np.float32)))
    pkb.add("fc1b", np.asarray(inputs["fc1_b"], np.float32)[:, None])
    pkb.add("fc2w", np.ascontiguousarray(np.asarray(inputs["fc2_w"], np.float32)))
    pkb.add("fc2b", np.asarray(inputs["fc2_b"], np.float32)[:, None])
    pkb.add("dstrel2", np.ascontiguousarray(
        _pad_rows(np.zeros(n2, np.float32), n2p, 1)[:, None]))
    pkb.add("l2rep", np.broadcast_to(np.concatenate(
        [row(s2_loc, n2p, 0, np.float32),
         row(np.zeros(n2), n2p, 1, np.float32)], axis=1), (P, 2 * n2p)))

    feed = {"xT": xT, "packa": pka.finish(), "packb": pkb.finish()}
    dims = dict(
        F=F, H1=H1, C=C, K=K, m2=m2, T2=T2,
        slots_a=tuple(sorted(pka.slots.items())),
        slots_b=tuple(sorted(pkb.slots.items())),
    )
    return feed, dims


def _build(dims):
    from concourse import bacc, bass, mybir, tile
    from concourse.masks import make_identity

    F = dims["F"]          # 128 input features
    H1 = dims["H1"]        # 4 heads, layer 1
    C = dims["C"]          # 64 channels per head
    D1 = H1 * C            # 256
    G1W = 2 * H1 + D1      # 264 = [ad(4) | as(4) | h(256)]
    G2W = 2 + C            # 66  = [ad2 | as2 | h2p]
    K = dims["K"]          # layer-1 chunks (edge slots per partition)
    KCH = D1 // P          # 2 contraction chunks over 256
    slots_a = dict(dims["slots_a"])
    slots_b = dict(dims["slots_b"])
    WA = max(b for _, b in slots_a.values())
    WB = max(b for _, b in slots_b.values())
    f32 = mybir.dt.float32
    bf16 = mybir.dt.bfloat16

    nc = bacc.Bacc("TRN2", target_bir_lowering=False, debug=False)

    xT_d = nc.declare_dram_parameter("xT", [F, K * P], bf16, isOutput=False)
    pa_d = nc.declare_dram_parameter("packa", [P, WA], f32, isOutput=False)
    pb_d = nc.declare_dram_parameter("packb", [P, WB], f32, isOutput=False)
    out_d = nc.declare_dram_parameter("out", [2, 1], f32, isOutput=True)

    EQ = mybir.AluOpType.is_equal
    MAX = mybir.AluOpType.max
    ADD = mybir.AluOpType.add
    MUL = mybir.AluOpType.mult
    Copy = mybir.ActivationFunctionType.Copy
    Exp = mybir.ActivationFunctionType.Exp
    Relu = mybir.ActivationFunctionType.Relu

    with tile.TileContext(nc) as tc:
        with (
            tc.tile_pool(name="const", bufs=1) as cp,
            tc.tile_pool(name="work", bufs=3) as wp,
            tc.tile_pool(name="fin", bufs=1) as fp,
            tc.tile_pool(name="psum", bufs=2, space="PSUM") as pp,
            tc.tile_pool(name="pgp", bufs=3, space="PSUM") as pgp,
            tc.tile_pool(name="acc", bufs=1, space="PSUM") as ap_,
        ):
            # ---------------- inputs into SBUF -----------------------------
            pka_s = cp.tile([P, WA], f32)
            nc.sync.dma_start(pka_s[:], pa_d[:])
            xT_s = cp.tile([F, K * P], bf16)
            nc.sync.dma_start(xT_s[:], xT_d[:])
            pkb_s = cp.tile([P, WB], f32)
            nc.gpsimd.dma_start(pkb_s[:], pb_d[:])

            def fsl(name, rows=P):
                if name in slots_a:
                    a, b = slots_a[name]
                    return pka_s[:rows, a:b]
                a, b = slots_b[name]
                return pkb_s[:rows, a:b]

            ident = cp.tile([P, P], f32)
            make_identity(nc, ident[:])
            iota_f = cp.tile([P, P], f32)
            nc.gpsimd.iota(
                iota_f[:], pattern=[[1, P]], base=0, channel_multiplier=0,
                allow_small_or_imprecise_dtypes=True,
            )
            iota_c = cp.tile([P, 1], f32)
            nc.gpsimd.iota(
                iota_c[:], pattern=[[0, 1]], base=0, channel_multiplier=1,
                allow_small_or_imprecise_dtypes=True,
            )

            # ---------------- W1 @ [Ad | As]  (K = 256, 2 chunks) ----------
            pwa = pp.tile([F, 2 * H1], f32, tag="mm")
            for k in range(KCH):
                nc.tensor.matmul(
                    out=pwa[:], lhsT=fsl(f"w1T{k}"), rhs=fsl(f"ablk{k}"),
                    start=(k == 0), stop=(k == KCH - 1),
                )
            rhs1 = cp.tile([F, G1W], bf16)
            nc.vector.tensor_copy(rhs1[:, : 2 * H1], pwa[:])
            nc.scalar.copy(rhs1[:, 2 * H1 :], fsl("w1"))

            # ---------------- layer-1 edge chunks --------------------------
            # chunk k: project 128 edge slots, form [h*exp(e) | exp(e)], and
            # accumulate + merge partition groups in PSUM via the one-hot
            # partition->slot matmul:  agg1 += pmap @ [msg | exp(e)]
            pmap = fp.tile([P, P], bf16)
            nc.vector.tensor_scalar(
                pmap[:], iota_f[:], fsl("pmapcol")[:, 0:1], None, EQ
            )
            agg1 = ap_.tile([P, D1 + H1], f32, tag="agg1")
            ad_part = fp.tile([P, H1], f32)
            admix = fp.tile([P, K * H1], f32)
            for k in range(K):
                pg = pgp.tile([P, G1W], f32, tag="pg")
                nc.tensor.matmul(
                    out=pg[:], lhsT=xT_s[:, k * P : (k + 1) * P],
                    rhs=rhs1[:], start=True, stop=True,
                )
                if k == 0:
                    # a_dst per partition from the self-loop rows, then fold
                    # in the validity mask for every chunk at once
                    nc.vector.tensor_copy(ad_part[:], pg[:, :H1])
                    nc.vector.tensor_tensor(
                        out=admix[:].rearrange("p (k h) -> p k h", h=H1),
                        in0=ad_part[:].rearrange("p (o h) -> p o h", o=1)
                        .to_broadcast([P, K, H1]),
                        in1=fsl("admask")[:].rearrange("p (k o) -> p k o", o=1)
                        .to_broadcast([P, K, H1]),
                        op=ADD,
                    )
                e = wp.tile([P, H1], f32, tag="e")
                nc.vector.tensor_tensor(
                    out=e[:], in0=pg[:, H1 : 2 * H1],
                    in1=admix[:, k * H1 : (k + 1) * H1], op=ADD,
                )
                es = wp.tile([P, H1], f32, tag="es")
                nc.vector.tensor_scalar_mul(es[:], e[:], 0.2)
                el = wp.tile([P, H1], f32, tag="el")
                nc.vector.tensor_tensor(out=el[:], in0=e[:], in1=es[:], op=MAX)
                pe = wp.tile([P, H1], f32, tag="pe")
                nc.scalar.activation(pe[:], el[:], Exp)
                msg = wp.tile([P, D1 + H1], bf16, tag="msg")
                nc.vector.tensor_tensor(
                    out=msg[:, :D1].rearrange("p (h c) -> p h c", c=C),
                    in0=pg[:, 2 * H1 :].rearrange("p (h c) -> p h c", c=C),
                    in1=pe[:].rearrange("p (h o) -> p h o", o=1).to_broadcast(
                        [P, H1, C]
                    ),
                    op=MUL,
                )
                nc.scalar.activation(msg[:, D1:], pe[:], Copy)
                nc.tensor.matmul(
                    out=agg1[:], lhsT=pmap[:], rhs=msg[:],
                    start=(k == 0), stop=(k == K - 1),
                )

            # layer-2 rhs [W2A2 | W2] per K-chunk + one-hots (no h1r dep)
            rhs2 = []
            for k in range(KCH):
                pwa2 = pp.tile([P, 2], f32, tag="mm")
                nc.tensor.matmul(
                    out=pwa2[:], lhsT=fsl("w2T", C)[:, k * P : (k + 1) * P],
                    rhs=fsl("a2", C), start=True, stop=True,
                )
                rhs2_k = cp.tile([P, G2W], f32, name=f"rhs2_{k}")
                nc.vector.tensor_copy(rhs2_k[:, :2], pwa2[:])
                nc.scalar.copy(rhs2_k[:, 2:], fsl(f"w2_{k}"))
                rhs2.append(rhs2_k)
            # layer-2 one-hots from the replicated [src2 | dstrel2] rows
            l2r = fsl("l2rep")
            st2 = fp.tile([P, P], bf16)
            nc.vector.tensor_scalar(st2[:], l2r[:, :P], iota_c[:, 0:1], None, EQ)
            em2t = fp.tile([P, P], f32)
            nc.vector.tensor_scalar(em2t[:], l2r[:, P:], iota_c[:, 0:1], None, EQ)

            # ---------------- layer-1 finalize: h1r = relu(num/den + b1) ---
            den1 = fp.tile([P, H1], f32)
            nc.vector.tensor_scalar_add(den1[:], agg1[:, D1:], 1e-16)
            rec1 = fp.tile([P, H1], f32)
            nc.vector.reciprocal(rec1[:], den1[:])
            h1t = fp.tile([P, D1], f32)
            nc.vector.tensor_tensor(
                out=h1t[:].rearrange("p (h c) -> p h c", c=C),
                in0=agg1[:, :D1].rearrange("p (h c) -> p h c", c=C),
                in1=rec1[:].rearrange("p (h o) -> p h o", o=1).to_broadcast(
                    [P, H1, C]
                ),
                op=MUL,
            )
            h1b = fp.tile([P, D1], f32)
            nc.vector.tensor_add(h1b[:], h1t[:], fsl("b1bc"))
            h1r = fp.tile([P, D1], f32)
            nc.scalar.activation(h1r[:], h1b[:], Relu)

            # ---------------- layer-2 projection: G2 = [ad2|as2|h2p] -------
            pg2 = ap_.tile([P, G2W], f32, tag="pg2")
            for k in range(KCH):
                ptr = pp.tile([P, P], f32, tag="mm")
                nc.tensor.transpose(
                    out=ptr[:], in_=h1r[:, k * P : (k + 1) * P], identity=ident[:]
                )
                h1rT_k = wp.tile([P, P], f32, tag=f"h1rTk{k}")
                nc.vector.tensor_copy(h1rT_k[:], ptr[:])
                nc.tensor.matmul(
                    out=pg2[:], lhsT=h1rT_k[:], rhs=rhs2[k][:],
                    start=(k == 0), stop=(k == KCH - 1),
                )
            g2sb = fp.tile([P, G2W], bf16)
            nc.scalar.copy(g2sb[:], pg2[:])
            g2ad = fp.tile([P, 1], f32)
            nc.vector.tensor_copy(g2ad[:], pg2[:, 0:1])

            # ------- layer-2 edge aggregation: fully on-chip (ball only) ---
            gs2_p = pp.tile([P, G2W], f32, tag="mm")
            nc.tensor.matmul(out=gs2_p[:], lhsT=st2[:], rhs=g2sb[:],
                             start=True, stop=False, skip_group_check=True)
            # accumulate the a_dst[ball] expansion straight onto the as2
            # column: gs2_p[:, 1] becomes e2 = as2[src] + ad2[dst]
            nc.tensor.matmul(out=gs2_p[:, 1:2], lhsT=em2t[:], rhs=g2ad[:],
                             start=False, stop=True, skip_group_check=True)
            es2 = fp.tile([P, 1], f32)
            nc.vector.tensor_scalar_mul(es2[:], gs2_p[:, 1:2], 0.2)
            el2 = fp.tile([P, 1], f32)
            nc.vector.tensor_tensor(
                out=el2[:], in0=gs2_p[:, 1:2], in1=es2[:], op=MAX
            )
            pe2 = fp.tile([P, 1], f32)
            nc.scalar.activation(pe2[:], el2[:], Exp)
            rhs2t = fp.tile([P, C + 1], bf16)
            nc.vector.tensor_tensor(
                out=rhs2t[:, :C], in0=gs2_p[:, 2:],
                in1=pe2[:].to_broadcast([P, C]), op=MUL,
            )
            nc.scalar.activation(rhs2t[:, C:], pe2[:], Copy)
            em2 = fp.tile([P, P], bf16)
            nc.vector.tensor_scalar(
                em2[:], iota_f[:], fsl("dstrel2")[:, 0:1], None, EQ
            )
            agg2 = ap_.tile([P, C + 1], f32, tag="agg2")
            nc.tensor.matmul(out=agg2[:], lhsT=em2[:], rhs=rhs2t[:],
                             start=True, stop=True)

            # ---------------- ball finalize + MLP --------------------------
            den2 = fp.tile([1, 1], f32)
            nc.vector.tensor_scalar_add(den2[:], agg2[0:1, C : C + 1], 1e-16)
            rec2 = fp.tile([1, 1], f32)
            nc.vector.reciprocal(rec2[:], den2[:])
            bf = fp.tile([1, C], f32)
            nc.scalar.activation(bf[:], agg2[0:1, :C], Copy, scale=rec2[:, 0:1])
            ptb = pp.tile([C, 1], f32, tag="mm")
            nc.tensor.transpose(out=ptb[:], in_=bf[:], identity=ident[0:1, 0:1])
            bfr = fp.tile([C, 1], f32)
            nc.scalar.activation(bfr[:], ptb[:], Relu, bias=fsl("b2col", C))

            pz = pp.tile([C // 2, 1], f32, tag="mm")
            nc.tensor.matmul(out=pz[:], lhsT=fsl("fc1w", C), rhs=bfr[:],
                             start=True, stop=True)
            zr = fp.tile([C // 2, 1], f32)
            nc.scalar.activation(zr[:], pz[:], Relu, bias=fsl("fc1b", C // 2))

            po = pp.tile([2, 1], f32, tag="mm")
            nc.tensor.matmul(out=po[:], lhsT=fsl("fc2w", C // 2), rhs=zr[:],
                             start=True, stop=True)
            osb = fp.tile([2, 1], f32)
            nc.vector.tensor_add(osb[:], po[:], fsl("fc2b", 2))
            nc.sync.dma_start(out_d[:], osb[:])

    nc.compile()
    return nc


def kernel(**inputs):
    from concourse.bass_utils import run_bass_kernel_spmd

    feed, dims = _host_preprocess(inputs)
    key = (dims["K"], dims["m2"], dims["T2"])
    if key not in _CACHE:
        _CACHE[key] = _build(dims)
    nc = _CACHE[key]

    n_cores = 8
    in_maps = [dict(feed) for _ in range(n_cores)]
    res = run_bass_kernel_spmd(nc, in_maps, core_ids=list(range(n_cores)))
    out = np.asarray(res.results[0]["out"], dtype=np.float32).reshape(2)
    return out
